# revision 27
# baseline (speedup 1.0000x reference)
"""Causal attention (B=4, S=2048, D=1024, fp32 in/out) on 8 Trainium2 cores.

Sharding: core c = (batch b = c//2, variant h = c%2). Each core computes the
attention output for 1024 of the 2048 query rows of one batch element.

Load balancing ("parity-slot" assignment): variant A owns even global
q-tiles (0,2,...,14), variant B owns odd (1,3,...,15). Slot i on every core
processes keys [0, CNT[i]*128) with CNT = (2,4,6,...,16), which dominates
both variants' causal needs (72 key-tiles vs the 68 minimum), so a single
NEFF serves all 8 cores; per-core differences are carried entirely by input
data (pre-sliced/pre-transposed X, per-strip diagonal mask blocks).

A consequence of the parity assignment: for score strip kt, ONLY the first
128-column slot block (slot JKT[kt] = kt//2) can have a nonzero causal
mask (diagonal for one variant, fully-masked padding for the other); all
later slot blocks are strictly below the diagonal for both variants. So
the mask input is just one [128,128] block per strip, and all remaining
eviction columns are plain copies, split across the ACT and DVE engines.

K/V are not recomputed per core: core (b, h) projects K^T/V only for its
own key half [h*1024, (h+1)*1024), and the pair exchanges halves with
chunked AllGathers over replica groups [[0,1],[2,3],[4,5],[6,7]] through
DRAM bounce buffers with partition-contiguous rows (4-8KB per partition,
fast DMA), pipelined so early key tiles land in SBUF while later
projection halves still compute.  No warm-up collective: the runtime's
collectives-init barrier occupies the CC stream until ~40us regardless.

Numerics: projections and AV run in bf16 (fp32 PSUM accum). Scores run in
fp8e4 (e4m3) with DoubleRow perf mode - each matmul contracts TWO 128-e
tiles into 64 psum partitions at 0.5 cycles/column, 2x bf16 throughput.
Q^T/K^T are cast fp32->fp8 at projection eviction; the 1/sqrt(1024) logit
scale is applied inside the exp activation (scale=1/32), so q/k stay at
full range where e4m3 quantization is benign. Measured end-to-end rel err
~1.3e-2 (vs 2e-2 budget).

DoubleRow cannot target PSUM partition offset 64 (invalid ISA), so the two
64-key groups of a strip go to separate psum regions at partition base 0;
the eviction writes group 1 to sT partitions 64:128 directly (engines
honor per-operand partition bases).

Tile-granularity dependencies: the Tile framework serializes readers
behind ALL writers of a tile, so every cross-phase tensor is split into
per-chunk tiles (qT per 512-q half, kT/v_sb per gather chunk, sT per
strip-group) to make phase overlap real: score strips 8..15 touch only
the qc=1 half of Q^T and start while the qc=0 half is still projecting.
"""

import numpy as np
from contextlib import ExitStack

import ml_dtypes

import concourse.bass as bass
import concourse.tile as tile
from concourse import bacc, mybir
from concourse.bass_utils import run_bass_kernel_spmd

P = 128
B, S, D = 4, 2048, 1024
NCORES = 8
DT = D // P      # 8 contraction tiles
ST = S // P      # 16 key tiles (global)
SLOC = S // 2    # 1024 local keys per core
SLT = SLOC // P  # 8 local key tiles
ET = D // P      # 8 output-feature tiles
QLOC = 1024      # query rows per core
QT = QLOC // P   # 8 local q tiles

G_A = tuple(range(0, ST, 2))         # variant A global q-tiles (slot order)
G_B = tuple(range(1, ST, 2))         # variant B
CNT = tuple(2 * i + 2 for i in range(QT))  # key tiles per slot (shared)
# Scores are computed transposed (S^T[k, q], keys on partitions).  Because
# CNT is ascending, the slots active for key-tile kt form a contiguous
# q-suffix starting at slot JKT[kt] = kt//2; WKT[kt] is that suffix width.
JKT = tuple(kt // 2 for kt in range(ST))
WKT = tuple((QT - j) * P for j in JKT)
NEG = -10000.0
INV_SQRT_D = 1.0 / 32.0
# Score strips 8..15 first (they only need the qc=1 half of Q^T), then
# 0..7.  Slots 0..3 finish at strip CNT[i]-1 in the second part; slots
# 4..7 need strips from both parts and all finish after strip 7.
STRIP_ORDER = tuple(range(8, ST)) + tuple(range(8))

F32 = mybir.dt.float32
BF16 = mybir.dt.bfloat16
F8 = mybir.dt.float8e4
DR = mybir.MatmulPerfMode.DoubleRow
EXP = mybir.ActivationFunctionType.Exp
COPY = mybir.ActivationFunctionType.Copy

REPLICA_GROUPS = [[0, 1], [2, 3], [4, 5], [6, 7]]


def _build(reps=1):
    nc = bacc.Bacc("TRN2", target_bir_lowering=False, debug=False,
                   num_devices=NCORES)
    xt_in = nc.dram_tensor("xt", [D, SLOC], BF16, kind="ExternalInput").ap()
    xqt_in = nc.dram_tensor("xqt", [D, QLOC], BF16, kind="ExternalInput").ap()
    wq_in = nc.dram_tensor("wq", [D, D], BF16, kind="ExternalInput").ap()
    wk_in = nc.dram_tensor("wk", [D, D], BF16, kind="ExternalInput").ap()
    wv_in = nc.dram_tensor("wv", [D, D], BF16, kind="ExternalInput").ap()
    # mask carries ST diagonal blocks + one trailing 128x128 identity used
    # by the PE mask-matmul.
    mask_in = nc.dram_tensor("mask", [P, (ST + 1) * P], BF16,
                             kind="ExternalInput").ap()
    out = nc.dram_tensor("out", [QLOC, D], F32, kind="ExternalOutput").ap()

    with tile.TileContext(nc) as tc, ExitStack() as ctx:
        persist = ctx.enter_context(tc.tile_pool(name="persist", bufs=1))
        # K^T per key chunk: [e%128, rank, et, key%512]; chunk kc covers
        # local key cols [kc*512,(kc+1)*512) of both ranks.
        kTc = [persist.tile([P, 2, ET, 512], F8, tag=f"kT{c}", name=f"kT{c}")
               for c in range(2)]
        # Q^T per 512-query half: [e%128, et, q%512]
        qTh = [persist.tile([P, ET, 512], F8, tag=f"qT{c}", name=f"qT{c}")
               for c in range(2)]
        # V per gather chunk: [k%128, rank, local kt%4, e]; chunk c covers
        # local key tiles [4c, 4c+4) of both ranks.
        vsbc = [persist.tile([P, 2, 4, D], BF16, tag=f"v{c}", name=f"v{c}")
                for c in range(2)]
        ones = persist.tile([P, 1], BF16, tag="ones")
        eye = persist.tile([P, P], BF16, tag="eye")
        # sTA (strips 8..15) must pre-exist phase A's xp pool: its strips
        # evict while the qc=0 Q projection still runs.  sTB is first
        # written only after qc=0 finishes, so it can reuse xp's space.
        sTA = persist.tile([P, 8, 512], F32, tag="sTA")

        for _rep in range(reps):
            _emit_body(nc, tc, _rep, xt_in, xqt_in, wq_in, wk_in, wv_in,
                       mask_in, out, kTc, qTh, vsbc, ones, eye, sTA)
    nc.compile()
    return nc


def _emit_body(nc, tc, rep, xt_in, xqt_in, wq_in, wk_in, wv_in, mask_in, out,
               kTc, qTh, vsbc, ones, eye, sTA):
    body = ExitStack()
    # Per-strip [128,128] diagonal mask blocks; tiny, prefetch all 16.
    mpool = body.enter_context(tc.tile_pool(name="m", bufs=ST))
    masks = {}

    def _load_mask(kt):
        m_t = mpool.tile([P, P], BF16, tag="m", name="m_t")
        nc.sync.dma_start(m_t, mask_in[:, kt * P:(kt + 1) * P])
        masks[kt] = m_t

    # psS pre-opened so its 4 PSUM banks are disjoint from psA's: the first
    # score strips run while the qc=0 Q projection still occupies psA
    # (otherwise psS allocation would serialize behind ALL of phase A).
    psS = body.enter_context(tc.tile_pool(name="psS", bufs=2, space="PSUM"))

    # ---------------- Phase A : projections + KV exchange ----------------
    with ExitStack() as pa:
        xp = pa.enter_context(tc.tile_pool(name="xp", bufs=1))
        dp = pa.enter_context(tc.tile_pool(name="dp", bufs=1, space="DRAM"))
        psA = pa.enter_context(tc.tile_pool(name="psA", bufs=4, space="PSUM"))

        nc.gpsimd.memset(ones[:], 1.0)
        nc.sync.dma_start(eye[:], mask_in[:, ST * P:(ST + 1) * P])

        # K-proj inputs (wk+xt) split across BOTH DMA queues so the first
        # matmul starts after ~0.5MB and per-dt delivery outpaces the PE.
        xt = xp.tile([P, DT, SLOC], BF16, tag="xt")
        wq_t = xp.tile([P, DT, D], BF16, tag="wq")
        wk_t = xp.tile([P, DT, D], BF16, tag="wk")
        wv_t = xp.tile([P, DT, D], BF16, tag="wv")
        xqt = xp.tile([P, DT, QLOC], BF16, tag="xqt")
        for dt in range(DT):
            nc.sync.dma_start(wk_t[:, dt, :], wk_in[dt * P:(dt + 1) * P, :])
            nc.scalar.dma_start(xt[:, dt, :], xt_in[dt * P:(dt + 1) * P, :])
        for dt in range(DT):
            nc.sync.dma_start(wv_t[:, dt, :], wv_in[dt * P:(dt + 1) * P, :])
        for dt in range(DT):
            nc.scalar.dma_start(xqt[:, dt, :], xqt_in[dt * P:(dt + 1) * P, :])
        for dt in range(DT):
            nc.scalar.dma_start(wq_t[:, dt, :], wq_in[dt * P:(dt + 1) * P, :])
        for kt in range(ST):
            _load_mask(kt)

        # Bounce layouts are partition-contiguous (4-8KB per partition
        # row), so stores/loads are single fast DMAs, not strided scatter.
        klocal = xp.tile([P, 2, ET, 512], F8, tag="klocal")
        vlocal = xp.tile([P, 2, 4, D], BF16, tag="vlocal")
        kbounce = [dp.tile([P, ET * 512], F8, tag="kb", name=f"kb{c}")
                   for c in range(2)]
        kgather = [dp.tile([2 * P, ET * 512], F8, tag="kg", name=f"kg{c}")
                   for c in range(2)]
        vbounce = [dp.tile([P, 4 * D], BF16, tag="vb", name=f"vb{c}")
                   for c in range(2)]
        vgather = [dp.tile([2 * P, 4 * D], BF16, tag="vg", name=f"vg{c}")
                   for c in range(2)]

        # K^T_loc[et, k] = sum_d Wk[d, et].T X_loc^T[d, k].  Halves are key
        # chunks (kc), so chunk kc's store+gather overlaps the other half's
        # matmuls; dt is the outer loop so matmuls start as slices land.
        for kc in range(2):
            for eh in range(2):  # psA holds 4 banks; 2 sub-rounds of 4 et
                ets = range(eh * 4, eh * 4 + 4)
                pss = {et: psA.tile([P, 512], F32, tag="ps", name="ps")
                       for et in ets}
                for dt in range(DT):
                    for et in ets:
                        nc.tensor.matmul(
                            pss[et][:], lhsT=wk_t[:, dt, et * P:(et + 1) * P],
                            rhs=xt[:, dt, kc * 512:(kc + 1) * 512],
                            start=(dt == 0), stop=(dt == DT - 1))
                for et in ets:
                    nc.vector.tensor_copy(klocal[:, kc, et, :], pss[et][:])
            nc.gpsimd.dma_start(
                kbounce[kc].rearrange("p (et k) -> p et k", et=ET),
                klocal[:, kc, :, :])
            nc.gpsimd.collective_compute(
                "AllGather", mybir.AluOpType.bypass,
                replica_groups=REPLICA_GROUPS,
                ins=[kbounce[kc].opt()], outs=[kgather[kc].opt()])
            # Gather-dependent loads go on the scalar queue (idle once the
            # inputs are streamed); nothing later must pass them.
            for r in range(2):
                nc.scalar.dma_start(
                    kTc[kc][:, r, :, :],
                    kgather[kc][r * P:(r + 1) * P, :].rearrange(
                        "p (et k) -> p et k", et=ET))

        # V_loc[kt, e] = sum_d X_loc^T[d, kt].T Wv[d, e]; each half (4 local
        # key tiles) is one store+gather chunk overlapping later compute.
        for half in range(2):
            for ec in range(2):  # sub-rounds of 4 (st x fixed ec)
                pss = [psA.tile([P, 512], F32, tag="ps", name="ps")
                       for _ in range(4)]
                for dt in range(DT):
                    for st in range(4):
                        nc.tensor.matmul(
                            pss[st][:],
                            lhsT=xt[:, dt, (half * 4 + st) * P:(half * 4 + st + 1) * P],
                            rhs=wv_t[:, dt, ec * 512:(ec + 1) * 512],
                            start=(dt == 0), stop=(dt == DT - 1))
                for st in range(4):
                    nc.vector.tensor_copy(
                        vlocal[:, half, st, ec * 512:(ec + 1) * 512], pss[st][:])
            nc.gpsimd.dma_start(
                vbounce[half].rearrange("p (st e) -> p st e", st=4),
                vlocal[:, half, :, :])
            nc.gpsimd.collective_compute(
                "AllGather", mybir.AluOpType.bypass,
                replica_groups=REPLICA_GROUPS,
                ins=[vbounce[half].opt()], outs=[vgather[half].opt()])
            for r in range(2):
                nc.scalar.dma_start(
                    vsbc[half][:, r, :, :],
                    vgather[half][r * P:(r + 1) * P, :].rearrange(
                        "p (st e) -> p st e", st=4))

        # Q^T[et, q] = sum_d Wq[d, et].T Xq^T[d, q].  Halves are q chunks,
        # qc=1 FIRST: score strips 8..15 touch only q-cols [512:1024), so
        # they start as soon as the qc=1 half is evicted, overlapping the
        # qc=0 half and hiding the Q->scores transition.
        for qc in (1, 0):
            for eh in range(2):
                ets = range(eh * 4, eh * 4 + 4)
                pss = {et: psA.tile([P, 512], F32, tag="ps", name="ps")
                       for et in ets}
                for dt in range(DT):
                    for et in ets:
                        nc.tensor.matmul(
                            pss[et][:], lhsT=wq_t[:, dt, et * P:(et + 1) * P],
                            rhs=xqt[:, dt, qc * 512:(qc + 1) * 512],
                            start=(dt == 0), stop=(dt == DT - 1))
                for et in ets:
                    nc.vector.tensor_copy(qTh[qc][:, et, :], pss[et][:])

    # ---------------- Phase B : attention (transposed scores) ----------
    # S^T[k, q] with keys on partitions, fp8 DoubleRow: each matmul
    # contracts an et PAIR into 64 psum partitions (one 64-key group).
    # exp(S^T) directly yields P^T -- the AV stationary operand.
    with body, ExitStack() as pb:
        stile = pb.enter_context(tc.tile_pool(name="st", bufs=1))
        # sTB (strips 0..7, full q range) reuses xp's space: first written
        # only after the qc=0 Q projection anyway.  sTA lives in persist.
        sTB = stile.tile([P, 8, QLOC], F32, tag="sTB")   # strips 0..7
        # per-slot P^T tiles so an early slot's AV only waits its own exp
        ptpool = pb.enter_context(tc.tile_pool(name="pt", bufs=QT))
        opool = pb.enter_context(tc.tile_pool(name="o", bufs=2))
        stpool = pb.enter_context(tc.tile_pool(name="stat", bufs=QT))
        psAV = pb.enter_context(tc.tile_pool(name="psAV", bufs=3, space="PSUM"))
        psRS = pb.enter_context(tc.tile_pool(name="psRS", bufs=1, space="PSUM"))
        rs = psRS.tile([P, QT], F32, tag="rs")           # rowsum, col per slot

        def _strip_dst(kt, s0, s1):
            # sT slice of strip kt covering strip-local cols [s0:s1),
            # returned as fn(kg) -> [64, s1-s0] AP at partitions kg*64.
            jq = JKT[kt] * P
            if kt >= 8:
                return lambda kg: sTA[kg * 64:(kg + 1) * 64, kt - 8,
                                      jq - 512 + s0:jq - 512 + s1]
            return lambda kg: sTB[kg * 64:(kg + 1) * 64, kt,
                                  jq + s0:jq + s1]

        pTs = {}
        for kt in STRIP_ORDER:
            jq = JKT[kt] * P
            w = WKT[kt]
            # chunks aligned to the global 512-q grid so each chunk's rhs
            # lives in exactly one qTh tile
            if jq >= 512:
                chunks = [(0, w)]
            else:
                chunks = [(0, 512 - jq), (512 - jq, w)]
            for c0, c1 in chunks:
                cw = c1 - c0
                qc = (jq + c0) // 512
                qoff = (jq + c0) - qc * 512
                ps = psS.tile([64, 2, 512], F32, tag="psS", name="ps")
                for kg in range(2):
                    lo = kt % 4 * P + kg * 64
                    for ep in range(ET // 2):
                        nc.tensor.matmul(
                            ps[:, kg, :cw],
                            lhsT=kTc[kt % 8 // 4][:, kt // 8,
                                                  2 * ep:2 * ep + 2,
                                                  lo:lo + 64],
                            rhs=qTh[qc][:, 2 * ep:2 * ep + 2,
                                        qoff:qoff + cw],
                            start=(ep == 0),
                            stop=(ep == ET // 2 - 1 and c0 != 0),
                            perf_mode=DR)
                    # Mask fold-in on the PE: only the strip's first 128
                    # cols (slot kt//2's diagonal/pad block) can be
                    # nonzero; eye.T @ mask accumulates mask rows
                    # [kg*64:(kg+1)*64] into the psum block, so evictions
                    # are pure copies with no DVE tensor_tensor.
                    if c0 == 0:
                        nc.tensor.matmul(
                            ps[:, kg, 0:P],
                            lhsT=eye[:, kg * 64:(kg + 1) * 64],
                            rhs=masks[kt][:], start=False, stop=True,
                            skip_group_check=True)
                # Copy-out split ACT (kg0) / DVE (kg1); kg1 lands on sT
                # partitions 64:128 directly (per-operand partition base).
                nc.scalar.activation(
                    _strip_dst(kt, c0, c1)(0), ps[:, 0, :cw], COPY)
                nc.vector.tensor_copy(
                    _strip_dst(kt, c0, c1)(1), ps[:, 1, :cw])
            # fire exp for every slot whose strips are all processed now:
            # slots 0..3 at their last strip (kt = CNT-1 in the second
            # part), slots 4..7 once strip 7 closes the second part.
            ready = [i for i in range(QT) if CNT[i] == kt + 1] if kt < 8 else []
            if kt == 7:
                ready += [i for i in range(QT) if CNT[i] > 8]
            for i in ready:
                pT_i = ptpool.tile([P, ST, P], BF16, tag="pt", name="pT_i")
                nc.scalar.activation(
                    pT_i[:, 0:min(CNT[i], 8), :],
                    sTB[:, 0:min(CNT[i], 8), i * P:(i + 1) * P],
                    EXP, scale=INV_SQRT_D)
                if CNT[i] > 8:
                    nc.scalar.activation(
                        pT_i[:, 8:CNT[i], :],
                        sTA[:, 0:CNT[i] - 8, (i - 4) * P:(i - 3) * P],
                        EXP, scale=INV_SQRT_D)
                pTs[i] = pT_i

        for i in range(QT):
            ck = CNT[i]
            pT_i = pTs[i]
            psavs = [psAV.tile([P, 512], F32, tag="psAV", name="psavs")
                     for _ in range(2)]
            for kt in range(ck):
                lhsT = pT_i[:, kt, :]
                nc.tensor.matmul(rs[:, i:i + 1], lhsT=lhsT, rhs=ones[:],
                                 start=(kt == 0), stop=(kt == ck - 1))
                for ec in range(2):
                    nc.tensor.matmul(
                        psavs[ec][:], lhsT=lhsT,
                        rhs=vsbc[kt % 8 // 4][:, kt // 8, kt % 4,
                                              ec * 512:(ec + 1) * 512],
                        start=(kt == 0), stop=(kt == ck - 1))

            recip = stpool.tile([P, 1], F32, tag="rc", name="recip")
            nc.vector.reciprocal(recip[:], rs[:, i:i + 1])
            for ec in range(2):
                o_t = opool.tile([P, 512], F32, tag="o", name="o_t")
                nc.scalar.activation(o_t[:], psavs[ec][:], COPY,
                                     scale=recip[:])
                nc.sync.dma_start(
                    out[i * P:(i + 1) * P, ec * 512:(ec + 1) * 512], o_t[:])


_COMPILED = None


def _get_compiled():
    global _COMPILED
    if _COMPILED is None:
        _COMPILED = _build()
    return _COMPILED


def _qrows(G):
    return np.concatenate([np.arange(g * P, (g + 1) * P) for g in G])


def _host_mask(G):
    # One [128,128] additive block per strip kt: the slot JKT[kt] block
    # (diagonal for one variant, full -NEG padding for the other; later
    # slots are strictly causal-active so their mask is identically 0).
    m = np.empty((P, (ST + 1) * P), np.float32)
    for kt in range(ST):
        key = kt * P + np.arange(P)[:, None]
        qpos = G[JKT[kt]] * P + np.arange(P)[None, :]
        m[:, kt * P:(kt + 1) * P] = np.where(
            key <= qpos, np.float32(0.0), np.float32(NEG))
    m[:, ST * P:] = np.eye(P, dtype=np.float32)
    return m.astype(ml_dtypes.bfloat16)


def _host_in_maps(X, Wq, Wk, Wv):
    bf = ml_dtypes.bfloat16
    X = np.asarray(X, np.float32)
    wq = np.asarray(Wq, np.float32).astype(bf)
    wk = np.asarray(Wk, np.float32).astype(bf)
    wv = np.asarray(Wv, np.float32).astype(bf)
    masks = {0: _host_mask(G_A), 1: _host_mask(G_B)}
    qr = {0: _qrows(G_A), 1: _qrows(G_B)}
    in_maps = []
    for c in range(NCORES):
        b, h = divmod(c, 2)
        Xb = X[b]
        in_maps.append({
            "xt": np.ascontiguousarray(Xb[h * SLOC:(h + 1) * SLOC].T).astype(bf),
            "xqt": np.ascontiguousarray(Xb[qr[h]].T).astype(bf),
            "wq": wq, "wk": wk, "wv": wv,
            "mask": masks[h],
        })
    return in_maps, qr


def kernel(X, Wq, Wk, Wv, _trace=False):
    nc = _get_compiled()
    in_maps, qr = _host_in_maps(X, Wq, Wk, Wv)
    res = run_bass_kernel_spmd(nc, in_maps, core_ids=list(range(NCORES)),
                               trace=_trace)
    O = np.empty((B, S, D), np.float32)
    for c in range(NCORES):
        b, h = divmod(c, 2)
        O[b, qr[h]] = res.results[c]["out"]
    if _trace:
        kernel._last_exec_time_ns = res.exec_time_ns
        kernel._last_results = res
    return O


# revision 33
# speedup vs baseline: 1.0159x; 1.0159x over previous
"""Causal attention (B=4, S=2048, D=1024, fp32 in/out) on 8 Trainium2 cores.

Sharding: core c = (batch b = c//2, variant h = c%2). Each core computes the
attention output for 1024 of the 2048 query rows of one batch element.

Load balancing ("parity-slot" assignment): variant A owns even global
q-tiles (0,2,...,14), variant B owns odd (1,3,...,15). Slot i on every core
processes keys [0, CNT[i]*128) with CNT = (2,4,6,...,16), which dominates
both variants' causal needs (72 key-tiles vs the 68 minimum), so a single
NEFF serves all 8 cores; per-core differences are carried entirely by input
data (pre-sliced/pre-transposed X, per-strip diagonal mask blocks).

A consequence of the parity assignment: for score strip kt, ONLY the first
128-column slot block (slot JKT[kt] = kt//2) can have a nonzero causal
mask (diagonal for one variant, fully-masked padding for the other); all
later slot blocks are strictly below the diagonal for both variants. So
the mask input is just one [128,128] block per strip, and all remaining
eviction columns are plain copies, split across the ACT and DVE engines.

K/V are not recomputed per core: core (b, h) projects K^T/V only for its
own key half [h*1024, (h+1)*1024), and the pair exchanges halves with
chunked AllGathers over replica groups [[0,1],[2,3],[4,5],[6,7]] through
DRAM bounce buffers with partition-contiguous rows (4-8KB per partition,
fast DMA), pipelined so early key tiles land in SBUF while later
projection halves still compute.  No warm-up collective: the runtime's
collectives-init barrier occupies the CC stream until ~40us regardless.

Numerics: projections and AV run in bf16 (fp32 PSUM accum). Scores run in
fp8e4 (e4m3) with DoubleRow perf mode - each matmul contracts TWO 128-e
tiles into 64 psum partitions at 0.5 cycles/column, 2x bf16 throughput.
Q^T/K^T are cast fp32->fp8 at projection eviction; the 1/sqrt(1024) logit
scale is applied inside the exp activation (scale=1/32), so q/k stay at
full range where e4m3 quantization is benign. Measured end-to-end rel err
~1.3e-2 (vs 2e-2 budget).

DoubleRow cannot target PSUM partition offset 64 (invalid ISA), so the two
64-key groups of a strip go to separate psum regions at partition base 0;
the eviction writes group 1 to sT partitions 64:128 directly (engines
honor per-operand partition bases).

Tile-granularity dependencies: the Tile framework serializes readers
behind ALL writers of a tile, so every cross-phase tensor is split into
per-chunk tiles (qT per 512-q half, kT/v_sb per gather chunk, sT per
strip-group) to make phase overlap real: score strips 8..15 touch only
the qc=1 half of Q^T and start while the qc=0 half is still projecting.
"""

import numpy as np
from contextlib import ExitStack

import ml_dtypes

import concourse.bass as bass
import concourse.tile as tile
from concourse import bacc, mybir
from concourse.bass_utils import run_bass_kernel_spmd

P = 128
B, S, D = 4, 2048, 1024
NCORES = 8
DT = D // P      # 8 contraction tiles
ST = S // P      # 16 key tiles (global)
SLOC = S // 2    # 1024 local keys per core
SLT = SLOC // P  # 8 local key tiles
ET = D // P      # 8 output-feature tiles
QLOC = 1024      # query rows per core
QT = QLOC // P   # 8 local q tiles

G_A = tuple(range(0, ST, 2))         # variant A global q-tiles (slot order)
G_B = tuple(range(1, ST, 2))         # variant B
CNT = tuple(2 * i + 2 for i in range(QT))  # key tiles per slot (shared)
# Scores are computed transposed (S^T[k, q], keys on partitions).  Because
# CNT is ascending, the slots active for key-tile kt form a contiguous
# q-suffix starting at slot JKT[kt] = kt//2; WKT[kt] is that suffix width.
JKT = tuple(kt // 2 for kt in range(ST))
WKT = tuple((QT - j) * P for j in JKT)
NEG = -10000.0
INV_SQRT_D = 1.0 / 32.0
# Score strips 8..15 first (they only need the qc=1 half of Q^T), then
# 0..7.  Slots 0..3 finish at strip CNT[i]-1 in the second part; slots
# 4..7 need strips from both parts and all finish after strip 7.
STRIP_ORDER = tuple(range(8, ST)) + tuple(range(8))

F32 = mybir.dt.float32
BF16 = mybir.dt.bfloat16
F8 = mybir.dt.float8e4
DR = mybir.MatmulPerfMode.DoubleRow
EXP = mybir.ActivationFunctionType.Exp
COPY = mybir.ActivationFunctionType.Copy

REPLICA_GROUPS = [[0, 1], [2, 3], [4, 5], [6, 7]]


def _build(reps=1):
    nc = bacc.Bacc("TRN2", target_bir_lowering=False, debug=False,
                   num_devices=NCORES)
    xt_in = nc.dram_tensor("xt", [D, SLOC], BF16, kind="ExternalInput").ap()
    xqt_in = nc.dram_tensor("xqt", [D, QLOC], BF16, kind="ExternalInput").ap()
    wq_in = nc.dram_tensor("wq", [D, D], BF16, kind="ExternalInput").ap()
    wk_in = nc.dram_tensor("wk", [D, D], BF16, kind="ExternalInput").ap()
    wv_in = nc.dram_tensor("wv", [D, D], BF16, kind="ExternalInput").ap()
    # mask carries ST diagonal blocks + one trailing 128x128 identity used
    # by the PE mask-matmul.
    mask_in = nc.dram_tensor("mask", [P, (ST + 1) * P], BF16,
                             kind="ExternalInput").ap()
    out = nc.dram_tensor("out", [QLOC, D], F32, kind="ExternalOutput").ap()

    with tile.TileContext(nc) as tc, ExitStack() as ctx:
        persist = ctx.enter_context(tc.tile_pool(name="persist", bufs=1))
        # K^T per key chunk: [e%128, rank, et, key%512]; chunk kc covers
        # local key cols [kc*512,(kc+1)*512) of both ranks.
        kTc = [persist.tile([P, 2, ET, 512], F8, tag=f"kT{c}", name=f"kT{c}")
               for c in range(2)]
        # Q^T per 512-query half: [e%128, et, q%512]
        qTh = [persist.tile([P, ET, 512], F8, tag=f"qT{c}", name=f"qT{c}")
               for c in range(2)]
        # V per gather chunk: [k%128, rank, local kt%4, e]; chunk c covers
        # local key tiles [4c, 4c+4) of both ranks.
        vsbc = [persist.tile([P, 2, 4, D], BF16, tag=f"v{c}", name=f"v{c}")
                for c in range(2)]
        ones = persist.tile([P, 1], BF16, tag="ones")
        eye = persist.tile([P, P], BF16, tag="eye")
        # sTA (strips 8..15) must pre-exist phase A's xp pool: its strips
        # evict while the qc=0 Q projection still runs.  sTB is first
        # written only after qc=0 finishes, so it can reuse xp's space.
        sTA = persist.tile([P, 8, 512], F32, tag="sTA")

        for _rep in range(reps):
            _emit_body(nc, tc, _rep, xt_in, xqt_in, wq_in, wk_in, wv_in,
                       mask_in, out, kTc, qTh, vsbc, ones, eye, sTA)
    nc.compile()
    return nc


def _emit_body(nc, tc, rep, xt_in, xqt_in, wq_in, wk_in, wv_in, mask_in, out,
               kTc, qTh, vsbc, ones, eye, sTA):
    body = ExitStack()
    # Per-strip [128,128] diagonal mask blocks; tiny, prefetch all 16.
    mpool = body.enter_context(tc.tile_pool(name="m", bufs=ST))
    masks = {}

    def _load_mask(kt):
        m_t = mpool.tile([P, P], BF16, tag="m", name="m_t")
        nc.sync.dma_start(m_t, mask_in[:, kt * P:(kt + 1) * P])
        masks[kt] = m_t

    # ---------------- Phase A : projections + KV exchange ----------------
    with ExitStack() as pa:
        xp = pa.enter_context(tc.tile_pool(name="xp", bufs=1))
        dp = pa.enter_context(tc.tile_pool(name="dp", bufs=1, space="DRAM"))
        # 8 interleaved psum chains: fewer (e.g. 4) raises the per-matmul
        # cost ~20% from psum-revisit hazards (measured).
        psA = pa.enter_context(tc.tile_pool(name="psA", bufs=8, space="PSUM"))

        nc.gpsimd.memset(ones[:], 1.0)
        nc.sync.dma_start(eye[:], mask_in[:, ST * P:(ST + 1) * P])

        # K-proj inputs (wk+xt) split across BOTH DMA queues so the first
        # matmul starts after ~0.5MB and per-dt delivery outpaces the PE.
        xt = xp.tile([P, DT, SLOC], BF16, tag="xt")
        wq_t = xp.tile([P, DT, D], BF16, tag="wq")
        wk_t = xp.tile([P, DT, D], BF16, tag="wk")
        wv_t = xp.tile([P, DT, D], BF16, tag="wv")
        xqt = xp.tile([P, DT, QLOC], BF16, tag="xqt")
        for dt in range(DT):
            nc.sync.dma_start(wk_t[:, dt, :], wk_in[dt * P:(dt + 1) * P, :])
            nc.scalar.dma_start(xt[:, dt, :], xt_in[dt * P:(dt + 1) * P, :])
        for dt in range(DT):
            nc.sync.dma_start(wv_t[:, dt, :], wv_in[dt * P:(dt + 1) * P, :])
        for dt in range(DT):
            nc.scalar.dma_start(xqt[:, dt, :], xqt_in[dt * P:(dt + 1) * P, :])
        for dt in range(DT):
            nc.scalar.dma_start(wq_t[:, dt, :], wq_in[dt * P:(dt + 1) * P, :])
        for kt in range(ST):
            _load_mask(kt)

        # Bounce layouts are partition-contiguous (4-8KB per partition
        # row), so stores/loads are single fast DMAs, not strided scatter.
        klocal = xp.tile([P, 2, ET, 512], F8, tag="klocal")
        vlocal = xp.tile([P, 2, 4, D], BF16, tag="vlocal")
        kbounce = [dp.tile([P, ET * 512], F8, tag="kb", name=f"kb{c}")
                   for c in range(2)]
        kgather = [dp.tile([2 * P, ET * 512], F8, tag="kg", name=f"kg{c}")
                   for c in range(2)]
        vbounce = [dp.tile([P, 4 * D], BF16, tag="vb", name=f"vb{c}")
                   for c in range(2)]
        vgather = [dp.tile([2 * P, 4 * D], BF16, tag="vg", name=f"vg{c}")
                   for c in range(2)]

        # K^T_loc[et, k] = sum_d Wk[d, et].T X_loc^T[d, k].  Halves are key
        # chunks (kc), so chunk kc's store+gather overlaps the other half's
        # matmuls; dt is the outer loop so matmuls start as slices land.
        for kc in range(2):
            pss = [psA.tile([P, 512], F32, tag="ps", name="ps")
                   for _ in range(ET)]
            for dt in range(DT):
                for et in range(ET):
                    nc.tensor.matmul(
                        pss[et][:], lhsT=wk_t[:, dt, et * P:(et + 1) * P],
                        rhs=xt[:, dt, kc * 512:(kc + 1) * 512],
                        start=(dt == 0), stop=(dt == DT - 1))
            for et in range(ET):
                nc.vector.tensor_copy(klocal[:, kc, et, :], pss[et][:])
            nc.gpsimd.dma_start(
                kbounce[kc].rearrange("p (et k) -> p et k", et=ET),
                klocal[:, kc, :, :])
            nc.gpsimd.collective_compute(
                "AllGather", mybir.AluOpType.bypass,
                replica_groups=REPLICA_GROUPS,
                ins=[kbounce[kc].opt()], outs=[kgather[kc].opt()])
            # Gather-dependent loads go on the scalar queue (idle once the
            # inputs are streamed); nothing later must pass them.
            for r in range(2):
                nc.scalar.dma_start(
                    kTc[kc][:, r, :, :],
                    kgather[kc][r * P:(r + 1) * P, :].rearrange(
                        "p (et k) -> p et k", et=ET))

        # V_loc[kt, e] = sum_d X_loc^T[d, kt].T Wv[d, e]; each half (4 local
        # key tiles) is one store+gather chunk overlapping later compute.
        for half in range(2):
            groups = [(st, ec) for st in range(4) for ec in range(2)]
            pss = [psA.tile([P, 512], F32, tag="ps", name="ps")
                   for _ in groups]
            for dt in range(DT):
                for gi, (st, ec) in enumerate(groups):
                    nc.tensor.matmul(
                        pss[gi][:],
                        lhsT=xt[:, dt, (half * 4 + st) * P:(half * 4 + st + 1) * P],
                        rhs=wv_t[:, dt, ec * 512:(ec + 1) * 512],
                        start=(dt == 0), stop=(dt == DT - 1))
            for gi, (st, ec) in enumerate(groups):
                nc.vector.tensor_copy(
                    vlocal[:, half, st, ec * 512:(ec + 1) * 512], pss[gi][:])
            nc.gpsimd.dma_start(
                vbounce[half].rearrange("p (st e) -> p st e", st=4),
                vlocal[:, half, :, :])
            nc.gpsimd.collective_compute(
                "AllGather", mybir.AluOpType.bypass,
                replica_groups=REPLICA_GROUPS,
                ins=[vbounce[half].opt()], outs=[vgather[half].opt()])
            for r in range(2):
                nc.scalar.dma_start(
                    vsbc[half][:, r, :, :],
                    vgather[half][r * P:(r + 1) * P, :].rearrange(
                        "p (st e) -> p st e", st=4))

        # Q^T[et, q] = sum_d Wq[d, et].T Xq^T[d, q].  Halves are q chunks,
        # qc=1 FIRST: score strips 8..15 touch only q-cols [512:1024), so
        # they start as soon as the qc=1 half is evicted, overlapping the
        # qc=0 half and hiding the Q->scores transition.
        for qc in (1, 0):
            pss = [psA.tile([P, 512], F32, tag="ps", name="ps")
                   for _ in range(ET)]
            for dt in range(DT):
                for et in range(ET):
                    nc.tensor.matmul(
                        pss[et][:], lhsT=wq_t[:, dt, et * P:(et + 1) * P],
                        rhs=xqt[:, dt, qc * 512:(qc + 1) * 512],
                        start=(dt == 0), stop=(dt == DT - 1))
            for et in range(ET):
                nc.vector.tensor_copy(qTh[qc][:, et, :], pss[et][:])

    # ---------------- Phase B : attention (transposed scores) ----------
    # S^T[k, q] with keys on partitions, fp8 DoubleRow: each matmul
    # contracts an et PAIR into 64 psum partitions (one 64-key group).
    # exp(S^T) directly yields P^T -- the AV stationary operand.
    with body, ExitStack() as pb:
        stile = pb.enter_context(tc.tile_pool(name="st", bufs=1))
        # sTB (strips 0..7, full q range) reuses xp's space: first written
        # only after the qc=0 Q projection anyway.  sTA lives in persist.
        sTB = stile.tile([P, 8, QLOC], F32, tag="sTB")   # strips 0..7
        # per-slot P^T tiles so an early slot's AV only waits its own exp
        ptpool = pb.enter_context(tc.tile_pool(name="pt", bufs=QT))
        opool = pb.enter_context(tc.tile_pool(name="o", bufs=2))
        stpool = pb.enter_context(tc.tile_pool(name="stat", bufs=QT))
        psS = pb.enter_context(tc.tile_pool(name="psS", bufs=2, space="PSUM"))
        psAV = pb.enter_context(tc.tile_pool(name="psAV", bufs=3, space="PSUM"))
        psRS = pb.enter_context(tc.tile_pool(name="psRS", bufs=1, space="PSUM"))
        rs = psRS.tile([P, QT], F32, tag="rs")           # rowsum, col per slot

        def _strip_dst(kt, s0, s1):
            # sT slice of strip kt covering strip-local cols [s0:s1),
            # returned as fn(kg) -> [64, s1-s0] AP at partitions kg*64.
            jq = JKT[kt] * P
            if kt >= 8:
                return lambda kg: sTA[kg * 64:(kg + 1) * 64, kt - 8,
                                      jq - 512 + s0:jq - 512 + s1]
            return lambda kg: sTB[kg * 64:(kg + 1) * 64, kt,
                                  jq + s0:jq + s1]

        pTs = {}
        for kt in STRIP_ORDER:
            jq = JKT[kt] * P
            w = WKT[kt]
            # chunks aligned to the global 512-q grid so each chunk's rhs
            # lives in exactly one qTh tile
            if jq >= 512:
                chunks = [(0, w)]
            else:
                chunks = [(0, 512 - jq), (512 - jq, w)]
            for c0, c1 in chunks:
                cw = c1 - c0
                qc = (jq + c0) // 512
                qoff = (jq + c0) - qc * 512
                ps = psS.tile([64, 2, 512], F32, tag="psS", name="ps")
                # ep-outer so the two kg chains interleave: back-to-back
                # matmuls never accumulate into the same psum region.
                for ep in range(ET // 2):
                    for kg in range(2):
                        lo = kt % 4 * P + kg * 64
                        nc.tensor.matmul(
                            ps[:, kg, :cw],
                            lhsT=kTc[kt % 8 // 4][:, kt // 8,
                                                  2 * ep:2 * ep + 2,
                                                  lo:lo + 64],
                            rhs=qTh[qc][:, 2 * ep:2 * ep + 2,
                                        qoff:qoff + cw],
                            start=(ep == 0),
                            stop=(ep == ET // 2 - 1 and c0 != 0),
                            perf_mode=DR)
                # Mask fold-in on the PE: only the strip's first 128 cols
                # (slot kt//2's diagonal/pad block) can be nonzero;
                # eye.T @ mask accumulates mask rows [kg*64:(kg+1)*64]
                # into the psum block, so evictions are pure copies with
                # no DVE tensor_tensor.
                if c0 == 0:
                    for kg in range(2):
                        nc.tensor.matmul(
                            ps[:, kg, 0:P],
                            lhsT=eye[:, kg * 64:(kg + 1) * 64],
                            rhs=masks[kt][:], start=False, stop=True,
                            skip_group_check=True)
                # Copy-out split ACT (kg0) / DVE (kg1); kg1 lands on sT
                # partitions 64:128 directly (per-operand partition base).
                nc.scalar.activation(
                    _strip_dst(kt, c0, c1)(0), ps[:, 0, :cw], COPY)
                nc.vector.tensor_copy(
                    _strip_dst(kt, c0, c1)(1), ps[:, 1, :cw])
            # fire exp for every slot whose strips are all processed now:
            # slots 0..3 at their last strip (kt = CNT-1 in the second
            # part), slots 4..7 once strip 7 closes the second part.
            ready = [i for i in range(QT) if CNT[i] == kt + 1] if kt < 8 else []
            if kt == 7:
                ready += [i for i in range(QT) if CNT[i] > 8]
            for i in ready:
                pT_i = ptpool.tile([P, ST, P], BF16, tag="pt", name="pT_i")
                nc.scalar.activation(
                    pT_i[:, 0:min(CNT[i], 8), :],
                    sTB[:, 0:min(CNT[i], 8), i * P:(i + 1) * P],
                    EXP, scale=INV_SQRT_D)
                if CNT[i] > 8:
                    nc.scalar.activation(
                        pT_i[:, 8:CNT[i], :],
                        sTA[:, 0:CNT[i] - 8, (i - 4) * P:(i - 3) * P],
                        EXP, scale=INV_SQRT_D)
                pTs[i] = pT_i

        for i in range(QT):
            ck = CNT[i]
            pT_i = pTs[i]
            psavs = [psAV.tile([P, 512], F32, tag="psAV", name="psavs")
                     for _ in range(2)]
            for kt in range(ck):
                lhsT = pT_i[:, kt, :]
                nc.tensor.matmul(rs[:, i:i + 1], lhsT=lhsT, rhs=ones[:],
                                 start=(kt == 0), stop=(kt == ck - 1))
                for ec in range(2):
                    nc.tensor.matmul(
                        psavs[ec][:], lhsT=lhsT,
                        rhs=vsbc[kt % 8 // 4][:, kt // 8, kt % 4,
                                              ec * 512:(ec + 1) * 512],
                        start=(kt == 0), stop=(kt == ck - 1))

            recip = stpool.tile([P, 1], F32, tag="rc", name="recip")
            nc.vector.reciprocal(recip[:], rs[:, i:i + 1])
            for ec in range(2):
                o_t = opool.tile([P, 512], F32, tag="o", name="o_t")
                nc.scalar.activation(o_t[:], psavs[ec][:], COPY,
                                     scale=recip[:])
                nc.sync.dma_start(
                    out[i * P:(i + 1) * P, ec * 512:(ec + 1) * 512], o_t[:])


_COMPILED = None


def _get_compiled():
    global _COMPILED
    if _COMPILED is None:
        _COMPILED = _build()
    return _COMPILED


def _qrows(G):
    return np.concatenate([np.arange(g * P, (g + 1) * P) for g in G])


def _host_mask(G):
    # One [128,128] additive block per strip kt: the slot JKT[kt] block
    # (diagonal for one variant, full -NEG padding for the other; later
    # slots are strictly causal-active so their mask is identically 0).
    m = np.empty((P, (ST + 1) * P), np.float32)
    for kt in range(ST):
        key = kt * P + np.arange(P)[:, None]
        qpos = G[JKT[kt]] * P + np.arange(P)[None, :]
        m[:, kt * P:(kt + 1) * P] = np.where(
            key <= qpos, np.float32(0.0), np.float32(NEG))
    m[:, ST * P:] = np.eye(P, dtype=np.float32)
    return m.astype(ml_dtypes.bfloat16)


def _host_in_maps(X, Wq, Wk, Wv):
    bf = ml_dtypes.bfloat16
    X = np.asarray(X, np.float32)
    wq = np.asarray(Wq, np.float32).astype(bf)
    wk = np.asarray(Wk, np.float32).astype(bf)
    wv = np.asarray(Wv, np.float32).astype(bf)
    masks = {0: _host_mask(G_A), 1: _host_mask(G_B)}
    qr = {0: _qrows(G_A), 1: _qrows(G_B)}
    in_maps = []
    for c in range(NCORES):
        b, h = divmod(c, 2)
        Xb = X[b]
        in_maps.append({
            "xt": np.ascontiguousarray(Xb[h * SLOC:(h + 1) * SLOC].T).astype(bf),
            "xqt": np.ascontiguousarray(Xb[qr[h]].T).astype(bf),
            "wq": wq, "wk": wk, "wv": wv,
            "mask": masks[h],
        })
    return in_maps, qr


def kernel(X, Wq, Wk, Wv, _trace=False):
    nc = _get_compiled()
    in_maps, qr = _host_in_maps(X, Wq, Wk, Wv)
    res = run_bass_kernel_spmd(nc, in_maps, core_ids=list(range(NCORES)),
                               trace=_trace)
    O = np.empty((B, S, D), np.float32)
    for c in range(NCORES):
        b, h = divmod(c, 2)
        O[b, qr[h]] = res.results[c]["out"]
    if _trace:
        kernel._last_exec_time_ns = res.exec_time_ns
        kernel._last_results = res
    return O


# revision 34
# speedup vs baseline: 1.1374x; 1.1197x over previous
"""Causal attention (B=4, S=2048, D=1024, fp32 in/out) on 8 Trainium2 cores.

Sharding: core c = (batch b = c//2, variant h = c%2). Each core computes the
attention output for 1024 of the 2048 query rows of one batch element.

Load balancing ("parity-slot" assignment): variant A owns even global
q-tiles (0,2,...,14), variant B owns odd (1,3,...,15). Slot i on every core
processes keys [0, CNT[i]*128) with CNT = (2,4,6,...,16), which dominates
both variants' causal needs (72 key-tiles vs the 68 minimum), so a single
NEFF serves all 8 cores; per-core differences are carried entirely by input
data (pre-sliced/pre-transposed X, per-strip diagonal mask blocks).

A consequence of the parity assignment: for score strip kt, ONLY the first
128-column slot block (slot JKT[kt] = kt//2) can have a nonzero causal
mask (diagonal for one variant, fully-masked padding for the other); all
later slot blocks are strictly below the diagonal for both variants. So
the mask input is just one [128,128] block per strip, and all remaining
eviction columns are plain copies, split across the ACT and DVE engines.

K/V are not recomputed per core: core (b, h) projects K^T/V only for its
own key half [h*1024, (h+1)*1024), and the pair exchanges halves with
chunked AllGathers over replica groups [[0,1],[2,3],[4,5],[6,7]] through
DRAM bounce buffers with partition-contiguous rows (4-8KB per partition,
fast DMA), pipelined so early key tiles land in SBUF while later
projection halves still compute.  No warm-up collective: the runtime's
collectives-init barrier occupies the CC stream until ~40us regardless.

Numerics: projections and AV run in bf16 (fp32 PSUM accum). Scores run in
fp8e4 (e4m3) with DoubleRow perf mode - each matmul contracts TWO 128-e
tiles into 64 psum partitions at 0.5 cycles/column, 2x bf16 throughput.
Q^T/K^T are cast fp32->fp8 at projection eviction; the 1/sqrt(1024) logit
scale is applied inside the exp activation (scale=1/32), so q/k stay at
full range where e4m3 quantization is benign. Measured end-to-end rel err
~1.3e-2 (vs 2e-2 budget).

DoubleRow cannot target PSUM partition offset 64 (invalid ISA), so the two
64-key groups of a strip go to separate psum regions at partition base 0;
the eviction writes group 1 to sT partitions 64:128 directly (engines
honor per-operand partition bases).

Tile-granularity dependencies: the Tile framework serializes readers
behind ALL writers of a tile, so every cross-phase tensor is split into
per-chunk tiles (qT per 512-q half, kT/v_sb per gather chunk, sT per
strip-group) to make phase overlap real: score strips 8..15 touch only
the qc=1 half of Q^T and start while the qc=0 half is still projecting.
"""

import numpy as np
from contextlib import ExitStack

import ml_dtypes

import concourse.bass as bass
import concourse.tile as tile
from concourse import bacc, mybir
from concourse.bass_utils import run_bass_kernel_spmd

P = 128
B, S, D = 4, 2048, 1024
NCORES = 8
DT = D // P      # 8 contraction tiles
ST = S // P      # 16 key tiles (global)
SLOC = S // 2    # 1024 local keys per core
SLT = SLOC // P  # 8 local key tiles
ET = D // P      # 8 output-feature tiles
QLOC = 1024      # query rows per core
QT = QLOC // P   # 8 local q tiles

G_A = tuple(range(0, ST, 2))         # variant A global q-tiles (slot order)
G_B = tuple(range(1, ST, 2))         # variant B
CNT = tuple(2 * i + 2 for i in range(QT))  # key tiles per slot (shared)
# Scores are computed transposed (S^T[k, q], keys on partitions).  Because
# CNT is ascending, the slots active for key-tile kt form a contiguous
# q-suffix starting at slot JKT[kt] = kt//2; WKT[kt] is that suffix width.
JKT = tuple(kt // 2 for kt in range(ST))
WKT = tuple((QT - j) * P for j in JKT)
NEG = -10000.0
INV_SQRT_D = 1.0 / 32.0
# Score strips 8..15 first (they only need the qc=1 half of Q^T), then
# 0..7.  Slots 0..3 finish at strip CNT[i]-1 in the second part; slots
# 4..7 need strips from both parts and all finish after strip 7.
STRIP_ORDER = tuple(range(8, ST)) + tuple(range(8))

F32 = mybir.dt.float32
BF16 = mybir.dt.bfloat16
F8 = mybir.dt.float8e4
DR = mybir.MatmulPerfMode.DoubleRow
EXP = mybir.ActivationFunctionType.Exp
COPY = mybir.ActivationFunctionType.Copy

REPLICA_GROUPS = [[0, 1], [2, 3], [4, 5], [6, 7]]


def _build(reps=1):
    nc = bacc.Bacc("TRN2", target_bir_lowering=False, debug=False,
                   num_devices=NCORES)
    xt_in = nc.dram_tensor("xt", [D, SLOC], BF16, kind="ExternalInput").ap()
    xqt_in = nc.dram_tensor("xqt", [D, QLOC], BF16, kind="ExternalInput").ap()
    wq_in = nc.dram_tensor("wq", [D, D], BF16, kind="ExternalInput").ap()
    wk_in = nc.dram_tensor("wk", [D, D], BF16, kind="ExternalInput").ap()
    wv_in = nc.dram_tensor("wv", [D, D], BF16, kind="ExternalInput").ap()
    mask_in = nc.dram_tensor("mask", [P, ST * P], BF16,
                             kind="ExternalInput").ap()
    out = nc.dram_tensor("out", [QLOC, D], F32, kind="ExternalOutput").ap()

    with tile.TileContext(nc) as tc, ExitStack() as ctx:
        persist = ctx.enter_context(tc.tile_pool(name="persist", bufs=1))
        # K^T per key chunk: [e%128, rank, et, key%512]; chunk kc covers
        # local key cols [kc*512,(kc+1)*512) of both ranks.
        kTc = [persist.tile([P, 2, ET, 512], F8, tag=f"kT{c}", name=f"kT{c}")
               for c in range(2)]
        # Q^T per 512-query half: [e%128, et, q%512]
        qTh = [persist.tile([P, ET, 512], F8, tag=f"qT{c}", name=f"qT{c}")
               for c in range(2)]
        # V per gather chunk: [k%128, rank, local kt%4, e]; chunk c covers
        # local key tiles [4c, 4c+4) of both ranks.
        vsbc = [persist.tile([P, 2, 4, D], BF16, tag=f"v{c}", name=f"v{c}")
                for c in range(2)]
        ones = persist.tile([P, 1], BF16, tag="ones")

        for _rep in range(reps):
            _emit_body(nc, tc, _rep, xt_in, xqt_in, wq_in, wk_in, wv_in,
                       mask_in, out, kTc, qTh, vsbc, ones)
    nc.compile()
    return nc


def _emit_body(nc, tc, rep, xt_in, xqt_in, wq_in, wk_in, wv_in, mask_in, out,
               kTc, qTh, vsbc, ones):
    body = ExitStack()
    # Per-strip [128,128] diagonal mask blocks; tiny, prefetch all 16.
    mpool = body.enter_context(tc.tile_pool(name="m", bufs=ST))
    masks = {}

    def _load_mask(kt):
        m_t = mpool.tile([P, P], BF16, tag="m", name="m_t")
        nc.sync.dma_start(m_t, mask_in[:, kt * P:(kt + 1) * P])
        masks[kt] = m_t

    # ---------------- Phase A : projections + KV exchange ----------------
    with ExitStack() as pa:
        xp = pa.enter_context(tc.tile_pool(name="xp", bufs=1))
        dp = pa.enter_context(tc.tile_pool(name="dp", bufs=1, space="DRAM"))
        psA = pa.enter_context(tc.tile_pool(name="psA", bufs=8, space="PSUM"))

        nc.gpsimd.memset(ones[:], 1.0)

        # K-proj inputs (wk+xt) split across BOTH DMA queues so the first
        # matmul starts after ~0.5MB and per-dt delivery outpaces the PE.
        xt = xp.tile([P, DT, SLOC], BF16, tag="xt")
        wq_t = xp.tile([P, DT, D], BF16, tag="wq")
        wk_t = xp.tile([P, DT, D], BF16, tag="wk")
        wv_t = xp.tile([P, DT, D], BF16, tag="wv")
        xqt = xp.tile([P, DT, QLOC], BF16, tag="xqt")
        for dt in range(DT):
            nc.sync.dma_start(wk_t[:, dt, :], wk_in[dt * P:(dt + 1) * P, :])
            nc.scalar.dma_start(xt[:, dt, :], xt_in[dt * P:(dt + 1) * P, :])
        for dt in range(DT):
            nc.sync.dma_start(wv_t[:, dt, :], wv_in[dt * P:(dt + 1) * P, :])
        for dt in range(DT):
            nc.scalar.dma_start(xqt[:, dt, :], xqt_in[dt * P:(dt + 1) * P, :])
        for dt in range(DT):
            nc.scalar.dma_start(wq_t[:, dt, :], wq_in[dt * P:(dt + 1) * P, :])
        for kt in range(ST):
            _load_mask(kt)

        # Bounce layouts are partition-contiguous (4-8KB per partition
        # row), so stores/loads are single fast DMAs, not strided scatter.
        klocal = xp.tile([P, 2, ET, 512], F8, tag="klocal")
        vlocal = xp.tile([P, 2, 4, D], BF16, tag="vlocal")
        kbounce = [dp.tile([P, ET * 512], F8, tag="kb", name=f"kb{c}")
                   for c in range(2)]
        kgather = [dp.tile([2 * P, ET * 512], F8, tag="kg", name=f"kg{c}")
                   for c in range(2)]
        vbounce = [dp.tile([P, 4 * D], BF16, tag="vb", name=f"vb{c}")
                   for c in range(2)]
        vgather = [dp.tile([2 * P, 4 * D], BF16, tag="vg", name=f"vg{c}")
                   for c in range(2)]

        # K^T_loc[et, k] = sum_d Wk[d, et].T X_loc^T[d, k].  Halves are key
        # chunks (kc), so chunk kc's store+gather overlaps the other half's
        # matmuls; dt is the outer loop so matmuls start as slices land.
        for kc in range(2):
            pss = [psA.tile([P, 512], F32, tag="ps", name="ps")
                   for _ in range(ET)]
            for dt in range(DT):
                for et in range(ET):
                    nc.tensor.matmul(
                        pss[et][:], lhsT=wk_t[:, dt, et * P:(et + 1) * P],
                        rhs=xt[:, dt, kc * 512:(kc + 1) * 512],
                        start=(dt == 0), stop=(dt == DT - 1))
            for et in range(ET):
                nc.vector.tensor_copy(klocal[:, kc, et, :], pss[et][:])
            nc.gpsimd.dma_start(
                kbounce[kc].rearrange("p (et k) -> p et k", et=ET),
                klocal[:, kc, :, :])
            nc.gpsimd.collective_compute(
                "AllGather", mybir.AluOpType.bypass,
                replica_groups=REPLICA_GROUPS,
                ins=[kbounce[kc].opt()], outs=[kgather[kc].opt()])
            # Gather-dependent loads go on the scalar queue (idle once the
            # inputs are streamed); nothing later must pass them.
            for r in range(2):
                nc.scalar.dma_start(
                    kTc[kc][:, r, :, :],
                    kgather[kc][r * P:(r + 1) * P, :].rearrange(
                        "p (et k) -> p et k", et=ET))

        # V_loc[kt, e] = sum_d X_loc^T[d, kt].T Wv[d, e]; each half (4 local
        # key tiles) is one store+gather chunk overlapping later compute.
        for half in range(2):
            groups = [(st, ec) for st in range(4) for ec in range(2)]
            pss = [psA.tile([P, 512], F32, tag="ps", name="ps")
                   for _ in groups]
            for dt in range(DT):
                for gi, (st, ec) in enumerate(groups):
                    nc.tensor.matmul(
                        pss[gi][:],
                        lhsT=xt[:, dt, (half * 4 + st) * P:(half * 4 + st + 1) * P],
                        rhs=wv_t[:, dt, ec * 512:(ec + 1) * 512],
                        start=(dt == 0), stop=(dt == DT - 1))
            for gi, (st, ec) in enumerate(groups):
                nc.vector.tensor_copy(
                    vlocal[:, half, st, ec * 512:(ec + 1) * 512], pss[gi][:])
            nc.gpsimd.dma_start(
                vbounce[half].rearrange("p (st e) -> p st e", st=4),
                vlocal[:, half, :, :])
            nc.gpsimd.collective_compute(
                "AllGather", mybir.AluOpType.bypass,
                replica_groups=REPLICA_GROUPS,
                ins=[vbounce[half].opt()], outs=[vgather[half].opt()])
            for r in range(2):
                nc.scalar.dma_start(
                    vsbc[half][:, r, :, :],
                    vgather[half][r * P:(r + 1) * P, :].rearrange(
                        "p (st e) -> p st e", st=4))

        # Q^T[et, q] = sum_d Wq[d, et].T Xq^T[d, q].  Halves are q chunks,
        # qc=1 FIRST: score strips 8..15 touch only q-cols [512:1024), so
        # they start as soon as the qc=1 half is evicted, overlapping the
        # qc=0 half and hiding the Q->scores transition.
        for qc in (1, 0):
            pss = [psA.tile([P, 512], F32, tag="ps", name="ps")
                   for _ in range(ET)]
            for dt in range(DT):
                for et in range(ET):
                    nc.tensor.matmul(
                        pss[et][:], lhsT=wq_t[:, dt, et * P:(et + 1) * P],
                        rhs=xqt[:, dt, qc * 512:(qc + 1) * 512],
                        start=(dt == 0), stop=(dt == DT - 1))
            for et in range(ET):
                nc.vector.tensor_copy(qTh[qc][:, et, :], pss[et][:])

    # ---------------- Phase B : attention (transposed scores) ----------
    # S^T[k, q] with keys on partitions, fp8 DoubleRow: each matmul
    # contracts an et PAIR into 64 psum partitions (one 64-key group).
    # exp(S^T) directly yields P^T -- the AV stationary operand.
    with body, ExitStack() as pb:
        stile = pb.enter_context(tc.tile_pool(name="st", bufs=1))
        # sT split: strips 8..15 span only q-cols [512:1024) -> 512 wide.
        sTA = stile.tile([P, 8, 512], F32, tag="sTA")    # strips 8..15
        sTB = stile.tile([P, 8, QLOC], F32, tag="sTB")   # strips 0..7
        # per-slot P^T tiles so an early slot's AV only waits its own exp
        ptpool = pb.enter_context(tc.tile_pool(name="pt", bufs=QT))
        opool = pb.enter_context(tc.tile_pool(name="o", bufs=2))
        stpool = pb.enter_context(tc.tile_pool(name="stat", bufs=QT))
        psS = pb.enter_context(tc.tile_pool(name="psS", bufs=2, space="PSUM"))
        psAV = pb.enter_context(tc.tile_pool(name="psAV", bufs=3, space="PSUM"))
        psRS = pb.enter_context(tc.tile_pool(name="psRS", bufs=1, space="PSUM"))
        rs = psRS.tile([P, QT], F32, tag="rs")           # rowsum, col per slot

        def _strip_dst(kt, s0, s1):
            # sT slice of strip kt covering strip-local cols [s0:s1),
            # returned as fn(kg) -> [64, s1-s0] AP at partitions kg*64.
            jq = JKT[kt] * P
            if kt >= 8:
                return lambda kg: sTA[kg * 64:(kg + 1) * 64, kt - 8,
                                      jq - 512 + s0:jq - 512 + s1]
            return lambda kg: sTB[kg * 64:(kg + 1) * 64, kt,
                                  jq + s0:jq + s1]

        pTs = {}
        for kt in STRIP_ORDER:
            jq = JKT[kt] * P
            w = WKT[kt]
            # chunks aligned to the global 512-q grid so each chunk's rhs
            # lives in exactly one qTh tile
            if jq >= 512:
                chunks = [(0, w)]
            else:
                chunks = [(0, 512 - jq), (512 - jq, w)]
            for c0, c1 in chunks:
                cw = c1 - c0
                qc = (jq + c0) // 512
                qoff = (jq + c0) - qc * 512
                ps = psS.tile([64, 2, 512], F32, tag="psS", name="ps")
                for kg in range(2):
                    lo = kt % 4 * P + kg * 64
                    for ep in range(ET // 2):
                        nc.tensor.matmul(
                            ps[:, kg, :cw],
                            lhsT=kTc[kt % 8 // 4][:, kt // 8,
                                                  2 * ep:2 * ep + 2,
                                                  lo:lo + 64],
                            rhs=qTh[qc][:, 2 * ep:2 * ep + 2,
                                        qoff:qoff + cw],
                            start=(ep == 0), stop=(ep == ET // 2 - 1),
                            perf_mode=DR)
                # Eviction: only the strip's first 128 cols can carry a
                # nonzero mask (diagonal/pad block of slot kt//2); they get
                # a DVE tensor_tensor add.  The rest are plain copies,
                # split ACT (kg0) / DVE (kg1) to halve eviction latency.
                # kg1 writes land on sT partitions 64:128 directly.
                r0 = c0
                if c0 == 0:
                    for kg in range(2):
                        nc.vector.tensor_tensor(
                            _strip_dst(kt, 0, P)(kg), ps[:, kg, 0:P],
                            masks[kt][kg * 64:(kg + 1) * 64, :],
                            op=mybir.AluOpType.add)
                    r0 = P
                if c1 > r0:
                    nc.scalar.activation(
                        _strip_dst(kt, r0, c1)(0), ps[:, 0, r0 - c0:cw], COPY)
                    nc.vector.tensor_copy(
                        _strip_dst(kt, r0, c1)(1), ps[:, 1, r0 - c0:cw])
            # fire exp for every slot whose strips are all processed now:
            # slots 0..3 at their last strip (kt = CNT-1 in the second
            # part), slots 4..7 once strip 7 closes the second part.
            ready = [i for i in range(QT) if CNT[i] == kt + 1] if kt < 8 else []
            if kt == 7:
                ready += [i for i in range(QT) if CNT[i] > 8]
            for i in ready:
                pT_i = ptpool.tile([P, ST, P], BF16, tag="pt", name="pT_i")
                nc.scalar.activation(
                    pT_i[:, 0:min(CNT[i], 8), :],
                    sTB[:, 0:min(CNT[i], 8), i * P:(i + 1) * P],
                    EXP, scale=INV_SQRT_D)
                if CNT[i] > 8:
                    nc.scalar.activation(
                        pT_i[:, 8:CNT[i], :],
                        sTA[:, 0:CNT[i] - 8, (i - 4) * P:(i - 3) * P],
                        EXP, scale=INV_SQRT_D)
                pTs[i] = pT_i

        for i in range(QT):
            ck = CNT[i]
            pT_i = pTs[i]
            psavs = [psAV.tile([P, 512], F32, tag="psAV", name="psavs")
                     for _ in range(2)]
            for kt in range(ck):
                lhsT = pT_i[:, kt, :]
                nc.tensor.matmul(rs[:, i:i + 1], lhsT=lhsT, rhs=ones[:],
                                 start=(kt == 0), stop=(kt == ck - 1))
                for ec in range(2):
                    nc.tensor.matmul(
                        psavs[ec][:], lhsT=lhsT,
                        rhs=vsbc[kt % 8 // 4][:, kt // 8, kt % 4,
                                              ec * 512:(ec + 1) * 512],
                        start=(kt == 0), stop=(kt == ck - 1))

            recip = stpool.tile([P, 1], F32, tag="rc", name="recip")
            nc.vector.reciprocal(recip[:], rs[:, i:i + 1])
            for ec in range(2):
                o_t = opool.tile([P, 512], F32, tag="o", name="o_t")
                nc.scalar.activation(o_t[:], psavs[ec][:], COPY,
                                     scale=recip[:])
                nc.sync.dma_start(
                    out[i * P:(i + 1) * P, ec * 512:(ec + 1) * 512], o_t[:])


_COMPILED = None


def _get_compiled():
    global _COMPILED
    if _COMPILED is None:
        _COMPILED = _build()
    return _COMPILED


def _qrows(G):
    return np.concatenate([np.arange(g * P, (g + 1) * P) for g in G])


def _host_mask(G):
    # One [128,128] additive block per strip kt: the slot JKT[kt] block
    # (diagonal for one variant, full -NEG padding for the other; later
    # slots are strictly causal-active so their mask is identically 0).
    m = np.empty((P, ST * P), np.float32)
    for kt in range(ST):
        key = kt * P + np.arange(P)[:, None]
        qpos = G[JKT[kt]] * P + np.arange(P)[None, :]
        m[:, kt * P:(kt + 1) * P] = np.where(
            key <= qpos, np.float32(0.0), np.float32(NEG))
    return m.astype(ml_dtypes.bfloat16)


def _host_in_maps(X, Wq, Wk, Wv):
    bf = ml_dtypes.bfloat16
    X = np.asarray(X, np.float32)
    wq = np.asarray(Wq, np.float32).astype(bf)
    wk = np.asarray(Wk, np.float32).astype(bf)
    wv = np.asarray(Wv, np.float32).astype(bf)
    masks = {0: _host_mask(G_A), 1: _host_mask(G_B)}
    qr = {0: _qrows(G_A), 1: _qrows(G_B)}
    in_maps = []
    for c in range(NCORES):
        b, h = divmod(c, 2)
        Xb = X[b]
        in_maps.append({
            "xt": np.ascontiguousarray(Xb[h * SLOC:(h + 1) * SLOC].T).astype(bf),
            "xqt": np.ascontiguousarray(Xb[qr[h]].T).astype(bf),
            "wq": wq, "wk": wk, "wv": wv,
            "mask": masks[h],
        })
    return in_maps, qr


def kernel(X, Wq, Wk, Wv, _trace=False):
    nc = _get_compiled()
    in_maps, qr = _host_in_maps(X, Wq, Wk, Wv)
    res = run_bass_kernel_spmd(nc, in_maps, core_ids=list(range(NCORES)),
                               trace=_trace)
    O = np.empty((B, S, D), np.float32)
    for c in range(NCORES):
        b, h = divmod(c, 2)
        O[b, qr[h]] = res.results[c]["out"]
    if _trace:
        kernel._last_exec_time_ns = res.exec_time_ns
        kernel._last_results = res
    return O


# revision 36
# speedup vs baseline: 1.1750x; 1.0330x over previous
"""Causal attention (B=4, S=2048, D=1024, fp32 in/out) on 8 Trainium2 cores.

Sharding: core c = (batch b = c//2, variant h = c%2). Each core computes the
attention output for 1024 of the 2048 query rows of one batch element.

Load balancing ("parity-slot" assignment): variant A owns even global
q-tiles (0,2,...,14), variant B owns odd (1,3,...,15). Slot i on every core
processes keys [0, CNT[i]*128) with CNT = (2,4,6,...,16), which dominates
both variants' causal needs (72 key-tiles vs the 68 minimum), so a single
NEFF serves all 8 cores; per-core differences are carried entirely by input
data (pre-sliced/pre-transposed X, per-strip diagonal mask blocks).

A consequence of the parity assignment: for score strip kt, ONLY the first
128-column slot block (slot JKT[kt] = kt//2) can have a nonzero causal
mask (diagonal for one variant, fully-masked padding for the other); all
later slot blocks are strictly below the diagonal for both variants. So
the mask input is just one [128,128] block per strip, and all remaining
eviction columns are plain copies, split across the ACT and DVE engines.

K/V are not recomputed per core: core (b, h) projects K^T/V only for its
own key half [h*1024, (h+1)*1024), and the pair exchanges halves with
chunked AllGathers over replica groups [[0,1],[2,3],[4,5],[6,7]] through
DRAM bounce buffers with partition-contiguous rows (4-8KB per partition,
fast DMA), pipelined so early key tiles land in SBUF while later
projection halves still compute.  No warm-up collective: the runtime's
collectives-init barrier occupies the CC stream until ~40us regardless.

Numerics: projections and AV run in bf16 (fp32 PSUM accum). Scores run in
fp8e4 (e4m3) with DoubleRow perf mode - each matmul contracts TWO 128-e
tiles into 64 psum partitions at 0.5 cycles/column, 2x bf16 throughput.
Q^T/K^T are cast fp32->fp8 at projection eviction; the 1/sqrt(1024) logit
scale is applied inside the exp activation (scale=1/32), so q/k stay at
full range where e4m3 quantization is benign. Measured end-to-end rel err
~1.3e-2 (vs 2e-2 budget).

DoubleRow cannot target PSUM partition offset 64 (invalid ISA), so the two
64-key groups of a strip go to separate psum regions at partition base 0;
the eviction writes group 1 to sT partitions 64:128 directly (engines
honor per-operand partition bases).

Tile-granularity dependencies: the Tile framework serializes readers
behind ALL writers of a tile, so every cross-phase tensor is split into
per-chunk tiles (qT per 512-q half, kT/v_sb per gather chunk, sT per
strip-group) to make phase overlap real: score strips 8..15 touch only
the qc=1 half of Q^T and start while the qc=0 half is still projecting.
"""

import numpy as np
from contextlib import ExitStack

import ml_dtypes

import concourse.bass as bass
import concourse.tile as tile
from concourse import bacc, mybir
from concourse.bass_utils import run_bass_kernel_spmd

P = 128
B, S, D = 4, 2048, 1024
NCORES = 8
DT = D // P      # 8 contraction tiles
ST = S // P      # 16 key tiles (global)
SLOC = S // 2    # 1024 local keys per core
SLT = SLOC // P  # 8 local key tiles
ET = D // P      # 8 output-feature tiles
QLOC = 1024      # query rows per core
QT = QLOC // P   # 8 local q tiles

G_A = tuple(range(0, ST, 2))         # variant A global q-tiles (slot order)
G_B = tuple(range(1, ST, 2))         # variant B
CNT = tuple(2 * i + 2 for i in range(QT))  # key tiles per slot (shared)
# Scores are computed transposed (S^T[k, q], keys on partitions).  Because
# CNT is ascending, the slots active for key-tile kt form a contiguous
# q-suffix starting at slot JKT[kt] = kt//2; WKT[kt] is that suffix width.
JKT = tuple(kt // 2 for kt in range(ST))
WKT = tuple((QT - j) * P for j in JKT)
NEG = -10000.0
INV_SQRT_D = 1.0 / 32.0
# Score strips 8..15 first (they only need the qc=1 half of Q^T), then
# 0..7.  Slots 0..3 finish at strip CNT[i]-1 in the second part; slots
# 4..7 need strips from both parts and all finish after strip 7.
STRIP_ORDER = tuple(range(8, ST)) + tuple(range(8))

F32 = mybir.dt.float32
BF16 = mybir.dt.bfloat16
F8 = mybir.dt.float8e4
DR = mybir.MatmulPerfMode.DoubleRow
EXP = mybir.ActivationFunctionType.Exp
COPY = mybir.ActivationFunctionType.Copy

REPLICA_GROUPS = [[0, 1], [2, 3], [4, 5], [6, 7]]


def _build(reps=1):
    nc = bacc.Bacc("TRN2", target_bir_lowering=False, debug=False,
                   num_devices=NCORES)
    xt_in = nc.dram_tensor("xt", [D, SLOC], BF16, kind="ExternalInput").ap()
    xqt_in = nc.dram_tensor("xqt", [D, QLOC], BF16, kind="ExternalInput").ap()
    wq_in = nc.dram_tensor("wq", [D, D], BF16, kind="ExternalInput").ap()
    wk_in = nc.dram_tensor("wk", [D, D], BF16, kind="ExternalInput").ap()
    wv_in = nc.dram_tensor("wv", [D, D], BF16, kind="ExternalInput").ap()
    mask_in = nc.dram_tensor("mask", [P, ST * P], BF16,
                             kind="ExternalInput").ap()
    out = nc.dram_tensor("out", [QLOC, D], F32, kind="ExternalOutput").ap()

    with tile.TileContext(nc) as tc, ExitStack() as ctx:
        persist = ctx.enter_context(tc.tile_pool(name="persist", bufs=1))
        # K^T per key chunk: [e%128, rank, et, key%512]; chunk kc covers
        # local key cols [kc*512,(kc+1)*512) of both ranks.
        kTc = [persist.tile([P, 2, ET, 512], F8, tag=f"kT{c}", name=f"kT{c}")
               for c in range(2)]
        # Q^T per 512-query half: [e%128, et, q%512]
        qTh = [persist.tile([P, ET, 512], F8, tag=f"qT{c}", name=f"qT{c}")
               for c in range(2)]
        # V per gather chunk: [k%128, rank, local kt%4, e]; chunk c covers
        # local key tiles [4c, 4c+4) of both ranks.
        vsbc = [persist.tile([P, 2, 4, D], BF16, tag=f"v{c}", name=f"v{c}")
                for c in range(2)]
        ones = persist.tile([P, 1], BF16, tag="ones")

        for _rep in range(reps):
            _emit_body(nc, tc, _rep, xt_in, xqt_in, wq_in, wk_in, wv_in,
                       mask_in, out, kTc, qTh, vsbc, ones)
    nc.compile()
    return nc


def _emit_body(nc, tc, rep, xt_in, xqt_in, wq_in, wk_in, wv_in, mask_in, out,
               kTc, qTh, vsbc, ones):
    body = ExitStack()
    # Per-strip [128,128] diagonal mask blocks; tiny, prefetch all 16.
    mpool = body.enter_context(tc.tile_pool(name="m", bufs=ST))
    masks = {}

    def _load_mask(kt):
        m_t = mpool.tile([P, P], BF16, tag="m", name="m_t")
        nc.sync.dma_start(m_t, mask_in[:, kt * P:(kt + 1) * P])
        masks[kt] = m_t

    # ---------------- Phase A : projections + KV exchange ----------------
    with ExitStack() as pa:
        xp = pa.enter_context(tc.tile_pool(name="xp", bufs=1))
        dp = pa.enter_context(tc.tile_pool(name="dp", bufs=1, space="DRAM"))
        psA = pa.enter_context(tc.tile_pool(name="psA", bufs=8, space="PSUM"))

        nc.gpsimd.memset(ones[:], 1.0)

        # K-proj inputs (wk+xt) split across BOTH DMA queues so the first
        # matmul starts after ~0.5MB and per-dt delivery outpaces the PE.
        xt = xp.tile([P, DT, SLOC], BF16, tag="xt")
        wq_t = xp.tile([P, DT, D], BF16, tag="wq")
        wk_t = xp.tile([P, DT, D], BF16, tag="wk")
        wv_t = xp.tile([P, DT, D], BF16, tag="wv")
        xqt = xp.tile([P, DT, QLOC], BF16, tag="xqt")
        for dt in range(DT):
            nc.sync.dma_start(wk_t[:, dt, :], wk_in[dt * P:(dt + 1) * P, :])
            nc.scalar.dma_start(xt[:, dt, :], xt_in[dt * P:(dt + 1) * P, :])
        for dt in range(DT):
            nc.sync.dma_start(wv_t[:, dt, :], wv_in[dt * P:(dt + 1) * P, :])
        for dt in range(DT):
            nc.scalar.dma_start(xqt[:, dt, :], xqt_in[dt * P:(dt + 1) * P, :])
        for dt in range(DT):
            nc.scalar.dma_start(wq_t[:, dt, :], wq_in[dt * P:(dt + 1) * P, :])
        for kt in range(ST):
            _load_mask(kt)

        # Bounce layouts are partition-contiguous (4-8KB per partition
        # row), so stores/loads are single fast DMAs, not strided scatter.
        klocal = xp.tile([P, 2, ET, 512], F8, tag="klocal")
        vlocal = xp.tile([P, 2, 4, D], BF16, tag="vlocal")
        kbounce = [dp.tile([P, ET * 512], F8, tag="kb", name=f"kb{c}")
                   for c in range(2)]
        kgather = [dp.tile([2 * P, ET * 512], F8, tag="kg", name=f"kg{c}")
                   for c in range(2)]
        vbounce = [dp.tile([P, 4 * D], BF16, tag="vb", name=f"vb{c}")
                   for c in range(2)]
        vgather = [dp.tile([2 * P, 4 * D], BF16, tag="vg", name=f"vg{c}")
                   for c in range(2)]

        # K^T_loc[et, k] = sum_d Wk[d, et].T X_loc^T[d, k].  Halves are key
        # chunks (kc), so chunk kc's store+gather overlaps the other half's
        # matmuls; dt is the outer loop so matmuls start as slices land.
        for kc in range(2):
            pss = [psA.tile([P, 512], F32, tag="ps", name="ps")
                   for _ in range(ET)]
            for dt in range(DT):
                for et in range(ET):
                    nc.tensor.matmul(
                        pss[et][:], lhsT=wk_t[:, dt, et * P:(et + 1) * P],
                        rhs=xt[:, dt, kc * 512:(kc + 1) * 512],
                        start=(dt == 0), stop=(dt == DT - 1))
            for et in range(ET):
                nc.vector.tensor_copy(klocal[:, kc, et, :], pss[et][:])
            nc.gpsimd.dma_start(
                kbounce[kc].rearrange("p (et k) -> p et k", et=ET),
                klocal[:, kc, :, :])
            nc.gpsimd.collective_compute(
                "AllGather", mybir.AluOpType.bypass,
                replica_groups=REPLICA_GROUPS,
                ins=[kbounce[kc].opt()], outs=[kgather[kc].opt()])
            # Gather-dependent loads go on the scalar queue (idle once the
            # inputs are streamed); nothing later must pass them.
            for r in range(2):
                nc.scalar.dma_start(
                    kTc[kc][:, r, :, :],
                    kgather[kc][r * P:(r + 1) * P, :].rearrange(
                        "p (et k) -> p et k", et=ET))

        # V_loc[kt, e] = sum_d X_loc^T[d, kt].T Wv[d, e]; each half (4 local
        # key tiles) is one store+gather chunk overlapping later compute.
        for half in range(2):
            groups = [(st, ec) for st in range(4) for ec in range(2)]
            pss = [psA.tile([P, 512], F32, tag="ps", name="ps")
                   for _ in groups]
            for dt in range(DT):
                for gi, (st, ec) in enumerate(groups):
                    nc.tensor.matmul(
                        pss[gi][:],
                        lhsT=xt[:, dt, (half * 4 + st) * P:(half * 4 + st + 1) * P],
                        rhs=wv_t[:, dt, ec * 512:(ec + 1) * 512],
                        start=(dt == 0), stop=(dt == DT - 1))
            for gi, (st, ec) in enumerate(groups):
                nc.vector.tensor_copy(
                    vlocal[:, half, st, ec * 512:(ec + 1) * 512], pss[gi][:])
            nc.gpsimd.dma_start(
                vbounce[half].rearrange("p (st e) -> p st e", st=4),
                vlocal[:, half, :, :])
            nc.gpsimd.collective_compute(
                "AllGather", mybir.AluOpType.bypass,
                replica_groups=REPLICA_GROUPS,
                ins=[vbounce[half].opt()], outs=[vgather[half].opt()])
            for r in range(2):
                nc.scalar.dma_start(
                    vsbc[half][:, r, :, :],
                    vgather[half][r * P:(r + 1) * P, :].rearrange(
                        "p (st e) -> p st e", st=4))

        # Q^T[et, q] = sum_d Wq[d, et].T Xq^T[d, q].  Halves are q chunks,
        # qc=1 FIRST: score strips 8..15 touch only q-cols [512:1024), so
        # they start as soon as the qc=1 half is evicted, overlapping the
        # qc=0 half and hiding the Q->scores transition.
        for qc in (1, 0):
            pss = [psA.tile([P, 512], F32, tag="ps", name="ps")
                   for _ in range(ET)]
            for dt in range(DT):
                for et in range(ET):
                    nc.tensor.matmul(
                        pss[et][:], lhsT=wq_t[:, dt, et * P:(et + 1) * P],
                        rhs=xqt[:, dt, qc * 512:(qc + 1) * 512],
                        start=(dt == 0), stop=(dt == DT - 1))
            for et in range(ET):
                nc.vector.tensor_copy(qTh[qc][:, et, :], pss[et][:])

    # ---------------- Phase B : attention (transposed scores) ----------
    # S^T[k, q] with keys on partitions, fp8 DoubleRow: each matmul
    # contracts an et PAIR into 64 psum partitions (one 64-key group).
    # exp(S^T) directly yields P^T -- the AV stationary operand.
    with body, ExitStack() as pb:
        stile = pb.enter_context(tc.tile_pool(name="st", bufs=1))
        # sT split: strips 8..15 span only q-cols [512:1024) -> 512 wide.
        sTA = stile.tile([P, 8, 512], F32, tag="sTA")    # strips 8..15
        sTB = stile.tile([P, 8, QLOC], F32, tag="sTB")   # strips 0..7
        # per-slot P^T tiles so an early slot's AV only waits its own exp
        ptpool = pb.enter_context(tc.tile_pool(name="pt", bufs=QT))
        opool = pb.enter_context(tc.tile_pool(name="o", bufs=2))
        stpool = pb.enter_context(tc.tile_pool(name="stat", bufs=QT))
        psS = pb.enter_context(tc.tile_pool(name="psS", bufs=2, space="PSUM"))
        psAV = pb.enter_context(tc.tile_pool(name="psAV", bufs=3, space="PSUM"))
        psRS = pb.enter_context(tc.tile_pool(name="psRS", bufs=1, space="PSUM"))
        rs = psRS.tile([P, QT], F32, tag="rs")           # rowsum, col per slot

        def _strip_dst(kt, s0, s1):
            # sT slice of strip kt covering strip-local cols [s0:s1),
            # returned as fn(kg) -> [64, s1-s0] AP at partitions kg*64.
            jq = JKT[kt] * P
            if kt >= 8:
                return lambda kg: sTA[kg * 64:(kg + 1) * 64, kt - 8,
                                      jq - 512 + s0:jq - 512 + s1]
            return lambda kg: sTB[kg * 64:(kg + 1) * 64, kt,
                                  jq + s0:jq + s1]

        pTs = {}
        for kt in STRIP_ORDER:
            jq = JKT[kt] * P
            w = WKT[kt]
            # chunks aligned to the global 512-q grid so each chunk's rhs
            # lives in exactly one qTh tile
            if jq >= 512:
                chunks = [(0, w)]
            else:
                chunks = [(0, 512 - jq), (512 - jq, w)]
            for c0, c1 in chunks:
                cw = c1 - c0
                qc = (jq + c0) // 512
                qoff = (jq + c0) - qc * 512
                ps = psS.tile([64, 2, 512], F32, tag="psS", name="ps")
                for kg in range(2):
                    lo = kt % 4 * P + kg * 64
                    for ep in range(ET // 2):
                        nc.tensor.matmul(
                            ps[:, kg, :cw],
                            lhsT=kTc[kt % 8 // 4][:, kt // 8,
                                                  2 * ep:2 * ep + 2,
                                                  lo:lo + 64],
                            rhs=qTh[qc][:, 2 * ep:2 * ep + 2,
                                        qoff:qoff + cw],
                            start=(ep == 0), stop=(ep == ET // 2 - 1),
                            perf_mode=DR)
                # Eviction: only the strip's first 128 cols can carry a
                # nonzero mask (diagonal/pad block of slot kt//2); they get
                # a DVE tensor_tensor add.  The rest are plain copies,
                # split ACT (kg0) / DVE (kg1) to halve eviction latency.
                # kg1 writes land on sT partitions 64:128 directly.
                # Full-width copy-out split ACT (kg0) / DVE (kg1); kg1
                # lands on sT partitions 64:128 directly.  The masked
                # first block then gets an in-place SBUF add on the
                # (otherwise idle) GpSimd engine, so the DVE carries only
                # one copy per chunk and psS keeps pace with the PE.
                nc.scalar.activation(
                    _strip_dst(kt, c0, c1)(0), ps[:, 0, :cw], COPY)
                nc.vector.tensor_copy(
                    _strip_dst(kt, c0, c1)(1), ps[:, 1, :cw])
                if c0 == 0:
                    for kg in range(2):
                        dst = _strip_dst(kt, 0, P)(kg)
                        nc.gpsimd.tensor_tensor(
                            dst, dst, masks[kt][kg * 64:(kg + 1) * 64, :],
                            op=mybir.AluOpType.add)
            # fire exp for every slot whose strips are all processed now:
            # slots 0..3 at their last strip (kt = CNT-1 in the second
            # part), slots 4..7 once strip 7 closes the second part.
            ready = [i for i in range(QT) if CNT[i] == kt + 1] if kt < 8 else []
            if kt == 7:
                ready += [i for i in range(QT) if CNT[i] > 8]
            for i in ready:
                pT_i = ptpool.tile([P, ST, P], BF16, tag="pt", name="pT_i")
                nc.scalar.activation(
                    pT_i[:, 0:min(CNT[i], 8), :],
                    sTB[:, 0:min(CNT[i], 8), i * P:(i + 1) * P],
                    EXP, scale=INV_SQRT_D)
                if CNT[i] > 8:
                    nc.scalar.activation(
                        pT_i[:, 8:CNT[i], :],
                        sTA[:, 0:CNT[i] - 8, (i - 4) * P:(i - 3) * P],
                        EXP, scale=INV_SQRT_D)
                pTs[i] = pT_i

        for i in range(QT):
            ck = CNT[i]
            pT_i = pTs[i]
            psavs = [psAV.tile([P, 512], F32, tag="psAV", name="psavs")
                     for _ in range(2)]
            for kt in range(ck):
                lhsT = pT_i[:, kt, :]
                nc.tensor.matmul(rs[:, i:i + 1], lhsT=lhsT, rhs=ones[:],
                                 start=(kt == 0), stop=(kt == ck - 1))
                for ec in range(2):
                    nc.tensor.matmul(
                        psavs[ec][:], lhsT=lhsT,
                        rhs=vsbc[kt % 8 // 4][:, kt // 8, kt % 4,
                                              ec * 512:(ec + 1) * 512],
                        start=(kt == 0), stop=(kt == ck - 1))

            recip = stpool.tile([P, 1], F32, tag="rc", name="recip")
            nc.vector.reciprocal(recip[:], rs[:, i:i + 1])
            for ec in range(2):
                o_t = opool.tile([P, 512], F32, tag="o", name="o_t")
                nc.scalar.activation(o_t[:], psavs[ec][:], COPY,
                                     scale=recip[:])
                nc.sync.dma_start(
                    out[i * P:(i + 1) * P, ec * 512:(ec + 1) * 512], o_t[:])


_COMPILED = None


def _get_compiled():
    global _COMPILED
    if _COMPILED is None:
        _COMPILED = _build()
    return _COMPILED


def _qrows(G):
    return np.concatenate([np.arange(g * P, (g + 1) * P) for g in G])


def _host_mask(G):
    # One [128,128] additive block per strip kt: the slot JKT[kt] block
    # (diagonal for one variant, full -NEG padding for the other; later
    # slots are strictly causal-active so their mask is identically 0).
    m = np.empty((P, ST * P), np.float32)
    for kt in range(ST):
        key = kt * P + np.arange(P)[:, None]
        qpos = G[JKT[kt]] * P + np.arange(P)[None, :]
        m[:, kt * P:(kt + 1) * P] = np.where(
            key <= qpos, np.float32(0.0), np.float32(NEG))
    return m.astype(ml_dtypes.bfloat16)


def _host_in_maps(X, Wq, Wk, Wv):
    bf = ml_dtypes.bfloat16
    X = np.asarray(X, np.float32)
    wq = np.asarray(Wq, np.float32).astype(bf)
    wk = np.asarray(Wk, np.float32).astype(bf)
    wv = np.asarray(Wv, np.float32).astype(bf)
    masks = {0: _host_mask(G_A), 1: _host_mask(G_B)}
    qr = {0: _qrows(G_A), 1: _qrows(G_B)}
    in_maps = []
    for c in range(NCORES):
        b, h = divmod(c, 2)
        Xb = X[b]
        in_maps.append({
            "xt": np.ascontiguousarray(Xb[h * SLOC:(h + 1) * SLOC].T).astype(bf),
            "xqt": np.ascontiguousarray(Xb[qr[h]].T).astype(bf),
            "wq": wq, "wk": wk, "wv": wv,
            "mask": masks[h],
        })
    return in_maps, qr


def kernel(X, Wq, Wk, Wv, _trace=False):
    nc = _get_compiled()
    in_maps, qr = _host_in_maps(X, Wq, Wk, Wv)
    res = run_bass_kernel_spmd(nc, in_maps, core_ids=list(range(NCORES)),
                               trace=_trace)
    O = np.empty((B, S, D), np.float32)
    for c in range(NCORES):
        b, h = divmod(c, 2)
        O[b, qr[h]] = res.results[c]["out"]
    if _trace:
        kernel._last_exec_time_ns = res.exec_time_ns
        kernel._last_results = res
    return O


# revision 37
# speedup vs baseline: 1.2622x; 1.0742x over previous
"""Causal attention (B=4, S=2048, D=1024, fp32 in/out) on 8 Trainium2 cores.

Sharding: core c = (batch b = c//2, variant h = c%2). Each core computes the
attention output for 1024 of the 2048 query rows of one batch element.

Load balancing ("parity-slot" assignment): variant A owns even global
q-tiles (0,2,...,14), variant B owns odd (1,3,...,15). Slot i on every core
processes keys [0, CNT[i]*128) with CNT = (2,4,6,...,16), which dominates
both variants' causal needs (72 key-tiles vs the 68 minimum), so a single
NEFF serves all 8 cores; per-core differences are carried entirely by input
data (pre-sliced/pre-transposed X, per-strip diagonal mask blocks).

A consequence of the parity assignment: for score strip kt, ONLY the first
128-column slot block (slot JKT[kt] = kt//2) can have a nonzero causal
mask (diagonal for one variant, fully-masked padding for the other); all
later slot blocks are strictly below the diagonal for both variants. So
the mask input is just one [128,128] block per strip, and all remaining
eviction columns are plain copies, split across the ACT and DVE engines.

K/V are not recomputed per core: core (b, h) projects K^T/V only for its
own key half [h*1024, (h+1)*1024), and the pair exchanges halves with
chunked AllGathers over replica groups [[0,1],[2,3],[4,5],[6,7]] through
DRAM bounce buffers with partition-contiguous rows (4-8KB per partition,
fast DMA), pipelined so early key tiles land in SBUF while later
projection halves still compute.  No warm-up collective: the runtime's
collectives-init barrier occupies the CC stream until ~40us regardless.

Numerics: projections and AV run in bf16 (fp32 PSUM accum). Scores run in
fp8e4 (e4m3) with DoubleRow perf mode - each matmul contracts TWO 128-e
tiles into 64 psum partitions at 0.5 cycles/column, 2x bf16 throughput.
Q^T/K^T are cast fp32->fp8 at projection eviction; the 1/sqrt(1024) logit
scale is applied inside the exp activation (scale=1/32), so q/k stay at
full range where e4m3 quantization is benign. Measured end-to-end rel err
~1.3e-2 (vs 2e-2 budget).

DoubleRow cannot target PSUM partition offset 64 (invalid ISA), so the two
64-key groups of a strip go to separate psum regions at partition base 0;
the eviction writes group 1 to sT partitions 64:128 directly (engines
honor per-operand partition bases).

Tile-granularity dependencies: the Tile framework serializes readers
behind ALL writers of a tile, so every cross-phase tensor is split into
per-chunk tiles (qT per 512-q half, kT/v_sb per gather chunk, sT per
strip-group) to make phase overlap real: score strips 8..15 touch only
the qc=1 half of Q^T and start while the qc=0 half is still projecting.
"""

import numpy as np
from contextlib import ExitStack

import ml_dtypes

import concourse.bass as bass
import concourse.tile as tile
from concourse import bacc, mybir
from concourse.bass_utils import run_bass_kernel_spmd

P = 128
B, S, D = 4, 2048, 1024
NCORES = 8
DT = D // P      # 8 contraction tiles
ST = S // P      # 16 key tiles (global)
SLOC = S // 2    # 1024 local keys per core
SLT = SLOC // P  # 8 local key tiles
ET = D // P      # 8 output-feature tiles
QLOC = 1024      # query rows per core
QT = QLOC // P   # 8 local q tiles

G_A = tuple(range(0, ST, 2))         # variant A global q-tiles (slot order)
G_B = tuple(range(1, ST, 2))         # variant B
CNT = tuple(2 * i + 2 for i in range(QT))  # key tiles per slot (shared)
# Scores are computed transposed (S^T[k, q], keys on partitions).  Because
# CNT is ascending, the slots active for key-tile kt form a contiguous
# q-suffix starting at slot JKT[kt] = kt//2; WKT[kt] is that suffix width.
JKT = tuple(kt // 2 for kt in range(ST))
WKT = tuple((QT - j) * P for j in JKT)
NEG = -10000.0
INV_SQRT_D = 1.0 / 32.0
# Score strips 8..15 first (they only need the qc=1 half of Q^T), then
# 0..7.  Slots 0..3 finish at strip CNT[i]-1 in the second part; slots
# 4..7 need strips from both parts and all finish after strip 7.
STRIP_ORDER = tuple(range(8, ST)) + tuple(range(8))

F32 = mybir.dt.float32
BF16 = mybir.dt.bfloat16
F8 = mybir.dt.float8e4
DR = mybir.MatmulPerfMode.DoubleRow
EXP = mybir.ActivationFunctionType.Exp
COPY = mybir.ActivationFunctionType.Copy

REPLICA_GROUPS = [[0, 1], [2, 3], [4, 5], [6, 7]]


def _build(reps=1):
    nc = bacc.Bacc("TRN2", target_bir_lowering=False, debug=False,
                   num_devices=NCORES)
    xt_in = nc.dram_tensor("xt", [D, SLOC], BF16, kind="ExternalInput").ap()
    xqt_in = nc.dram_tensor("xqt", [D, QLOC], BF16, kind="ExternalInput").ap()
    wq_in = nc.dram_tensor("wq", [D, D], BF16, kind="ExternalInput").ap()
    wk_in = nc.dram_tensor("wk", [D, D], BF16, kind="ExternalInput").ap()
    wv_in = nc.dram_tensor("wv", [D, D], BF16, kind="ExternalInput").ap()
    mask_in = nc.dram_tensor("mask", [P, ST * P], BF16,
                             kind="ExternalInput").ap()
    out = nc.dram_tensor("out", [QLOC, D], F32, kind="ExternalOutput").ap()

    with tile.TileContext(nc) as tc, ExitStack() as ctx:
        persist = ctx.enter_context(tc.tile_pool(name="persist", bufs=1))
        # K^T per key chunk: [e%128, rank, et, key%512]; chunk kc covers
        # local key cols [kc*512,(kc+1)*512) of both ranks.
        kTc = [persist.tile([P, 2, ET, 512], F8, tag=f"kT{c}", name=f"kT{c}")
               for c in range(2)]
        # Q^T per 512-query half: [e%128, et, q%512]
        qTh = [persist.tile([P, ET, 512], F8, tag=f"qT{c}", name=f"qT{c}")
               for c in range(2)]
        # V per gather chunk: [k%128, rank, local kt%4, e]; chunk c covers
        # local key tiles [4c, 4c+4) of both ranks.
        vsbc = [persist.tile([P, 2, 4, D], BF16, tag=f"v{c}", name=f"v{c}")
                for c in range(2)]
        ones = persist.tile([P, 1], BF16, tag="ones")

        for _rep in range(reps):
            _emit_body(nc, tc, _rep, xt_in, xqt_in, wq_in, wk_in, wv_in,
                       mask_in, out, kTc, qTh, vsbc, ones)
    nc.compile()
    return nc


def _emit_body(nc, tc, rep, xt_in, xqt_in, wq_in, wk_in, wv_in, mask_in, out,
               kTc, qTh, vsbc, ones):
    body = ExitStack()
    # Per-strip [128,128] diagonal mask blocks; tiny, prefetch all 16.
    mpool = body.enter_context(tc.tile_pool(name="m", bufs=ST))
    masks = {}

    def _load_mask(kt):
        m_t = mpool.tile([P, P], BF16, tag="m", name="m_t")
        nc.sync.dma_start(m_t, mask_in[:, kt * P:(kt + 1) * P])
        masks[kt] = m_t

    # ---------------- Phase A : projections + KV exchange ----------------
    with ExitStack() as pa:
        xp = pa.enter_context(tc.tile_pool(name="xp", bufs=1))
        dp = pa.enter_context(tc.tile_pool(name="dp", bufs=1, space="DRAM"))
        psA = pa.enter_context(tc.tile_pool(name="psA", bufs=8, space="PSUM"))

        nc.gpsimd.memset(ones[:], 1.0)

        # K-proj inputs (wk+xt) split across BOTH DMA queues so the first
        # matmul starts after ~0.5MB and per-dt delivery outpaces the PE.
        xt = xp.tile([P, DT, SLOC], BF16, tag="xt")
        wq_t = xp.tile([P, DT, D], BF16, tag="wq")
        wk_t = xp.tile([P, DT, D], BF16, tag="wk")
        wv_t = xp.tile([P, DT, D], BF16, tag="wv")
        xqt = xp.tile([P, DT, QLOC], BF16, tag="xqt")
        for dt in range(DT):
            nc.sync.dma_start(wk_t[:, dt, :], wk_in[dt * P:(dt + 1) * P, :])
            nc.scalar.dma_start(xt[:, dt, :], xt_in[dt * P:(dt + 1) * P, :])
        for dt in range(DT):
            nc.sync.dma_start(wv_t[:, dt, :], wv_in[dt * P:(dt + 1) * P, :])
        for dt in range(DT):
            nc.scalar.dma_start(xqt[:, dt, :], xqt_in[dt * P:(dt + 1) * P, :])
        for dt in range(DT):
            nc.scalar.dma_start(wq_t[:, dt, :], wq_in[dt * P:(dt + 1) * P, :])
        for kt in range(ST):
            _load_mask(kt)

        # Bounce layouts are partition-contiguous (4-8KB per partition
        # row), so stores/loads are single fast DMAs, not strided scatter.
        klocal = xp.tile([P, 2, ET, 512], F8, tag="klocal")
        vlocal = xp.tile([P, 2, 4, D], BF16, tag="vlocal")
        kbounce = [dp.tile([P, ET * 512], F8, tag="kb", name=f"kb{c}")
                   for c in range(2)]
        kgather = [dp.tile([2 * P, ET * 512], F8, tag="kg", name=f"kg{c}")
                   for c in range(2)]
        vbounce = [dp.tile([P, 4 * D], BF16, tag="vb", name=f"vb{c}")
                   for c in range(2)]
        vgather = [dp.tile([2 * P, 4 * D], BF16, tag="vg", name=f"vg{c}")
                   for c in range(2)]

        # K^T_loc[et, k] = sum_d Wk[d, et].T X_loc^T[d, k].  Halves are key
        # chunks (kc), so chunk kc's store+gather overlaps the other half's
        # matmuls; dt is the outer loop so matmuls start as slices land.
        for kc in range(2):
            pss = [psA.tile([P, 512], F32, tag="ps", name="ps")
                   for _ in range(ET)]
            for dt in range(DT):
                for et in range(ET):
                    nc.tensor.matmul(
                        pss[et][:], lhsT=wk_t[:, dt, et * P:(et + 1) * P],
                        rhs=xt[:, dt, kc * 512:(kc + 1) * 512],
                        start=(dt == 0), stop=(dt == DT - 1))
            for et in range(ET):
                nc.vector.tensor_copy(klocal[:, kc, et, :], pss[et][:])
            nc.gpsimd.dma_start(
                kbounce[kc].rearrange("p (et k) -> p et k", et=ET),
                klocal[:, kc, :, :])
            nc.gpsimd.collective_compute(
                "AllGather", mybir.AluOpType.bypass,
                replica_groups=REPLICA_GROUPS,
                ins=[kbounce[kc].opt()], outs=[kgather[kc].opt()])
            # Gather-dependent loads go on the sync queue: the scalar
            # queue carries the phase-B ACT psS eviction copies, which a
            # waiting DMA trigger would head-of-line block behind the
            # V collective.
            for r in range(2):
                nc.sync.dma_start(
                    kTc[kc][:, r, :, :],
                    kgather[kc][r * P:(r + 1) * P, :].rearrange(
                        "p (et k) -> p et k", et=ET))

        # V_loc[kt, e] = sum_d X_loc^T[d, kt].T Wv[d, e]; each half (4 local
        # key tiles) is one store+gather chunk overlapping later compute.
        for half in range(2):
            groups = [(st, ec) for st in range(4) for ec in range(2)]
            pss = [psA.tile([P, 512], F32, tag="ps", name="ps")
                   for _ in groups]
            for dt in range(DT):
                for gi, (st, ec) in enumerate(groups):
                    nc.tensor.matmul(
                        pss[gi][:],
                        lhsT=xt[:, dt, (half * 4 + st) * P:(half * 4 + st + 1) * P],
                        rhs=wv_t[:, dt, ec * 512:(ec + 1) * 512],
                        start=(dt == 0), stop=(dt == DT - 1))
            for gi, (st, ec) in enumerate(groups):
                nc.vector.tensor_copy(
                    vlocal[:, half, st, ec * 512:(ec + 1) * 512], pss[gi][:])
            nc.gpsimd.dma_start(
                vbounce[half].rearrange("p (st e) -> p st e", st=4),
                vlocal[:, half, :, :])
            nc.gpsimd.collective_compute(
                "AllGather", mybir.AluOpType.bypass,
                replica_groups=REPLICA_GROUPS,
                ins=[vbounce[half].opt()], outs=[vgather[half].opt()])
            for r in range(2):
                nc.sync.dma_start(
                    vsbc[half][:, r, :, :],
                    vgather[half][r * P:(r + 1) * P, :].rearrange(
                        "p (st e) -> p st e", st=4))

        # Q^T[et, q] = sum_d Wq[d, et].T Xq^T[d, q].  Halves are q chunks,
        # qc=1 FIRST: score strips 8..15 touch only q-cols [512:1024), so
        # they start as soon as the qc=1 half is evicted, overlapping the
        # qc=0 half and hiding the Q->scores transition.
        for qc in (1, 0):
            pss = [psA.tile([P, 512], F32, tag="ps", name="ps")
                   for _ in range(ET)]
            for dt in range(DT):
                for et in range(ET):
                    nc.tensor.matmul(
                        pss[et][:], lhsT=wq_t[:, dt, et * P:(et + 1) * P],
                        rhs=xqt[:, dt, qc * 512:(qc + 1) * 512],
                        start=(dt == 0), stop=(dt == DT - 1))
            for et in range(ET):
                nc.vector.tensor_copy(qTh[qc][:, et, :], pss[et][:])

    # ---------------- Phase B : attention (transposed scores) ----------
    # S^T[k, q] with keys on partitions, fp8 DoubleRow: each matmul
    # contracts an et PAIR into 64 psum partitions (one 64-key group).
    # exp(S^T) directly yields P^T -- the AV stationary operand.
    with body, ExitStack() as pb:
        stile = pb.enter_context(tc.tile_pool(name="st", bufs=1))
        # sT split: strips 8..15 span only q-cols [512:1024) -> 512 wide.
        sTA = stile.tile([P, 8, 512], F32, tag="sTA")    # strips 8..15
        sTB = stile.tile([P, 8, QLOC], F32, tag="sTB")   # strips 0..7
        # per-slot P^T tiles so an early slot's AV only waits its own exp
        ptpool = pb.enter_context(tc.tile_pool(name="pt", bufs=QT))
        opool = pb.enter_context(tc.tile_pool(name="o", bufs=2))
        stpool = pb.enter_context(tc.tile_pool(name="stat", bufs=QT))
        psS = pb.enter_context(tc.tile_pool(name="psS", bufs=2, space="PSUM"))
        psAV = pb.enter_context(tc.tile_pool(name="psAV", bufs=3, space="PSUM"))
        psRS = pb.enter_context(tc.tile_pool(name="psRS", bufs=1, space="PSUM"))
        rs = psRS.tile([P, QT], F32, tag="rs")           # rowsum, col per slot

        def _strip_dst(kt, s0, s1):
            # sT slice of strip kt covering strip-local cols [s0:s1),
            # returned as fn(kg) -> [64, s1-s0] AP at partitions kg*64.
            jq = JKT[kt] * P
            if kt >= 8:
                return lambda kg: sTA[kg * 64:(kg + 1) * 64, kt - 8,
                                      jq - 512 + s0:jq - 512 + s1]
            return lambda kg: sTB[kg * 64:(kg + 1) * 64, kt,
                                  jq + s0:jq + s1]

        pTs = {}
        for kt in STRIP_ORDER:
            jq = JKT[kt] * P
            w = WKT[kt]
            # chunks aligned to the global 512-q grid so each chunk's rhs
            # lives in exactly one qTh tile
            if jq >= 512:
                chunks = [(0, w)]
            else:
                chunks = [(0, 512 - jq), (512 - jq, w)]
            for c0, c1 in chunks:
                cw = c1 - c0
                qc = (jq + c0) // 512
                qoff = (jq + c0) - qc * 512
                ps = psS.tile([64, 2, 512], F32, tag="psS", name="ps")
                for kg in range(2):
                    lo = kt % 4 * P + kg * 64
                    for ep in range(ET // 2):
                        nc.tensor.matmul(
                            ps[:, kg, :cw],
                            lhsT=kTc[kt % 8 // 4][:, kt // 8,
                                                  2 * ep:2 * ep + 2,
                                                  lo:lo + 64],
                            rhs=qTh[qc][:, 2 * ep:2 * ep + 2,
                                        qoff:qoff + cw],
                            start=(ep == 0), stop=(ep == ET // 2 - 1),
                            perf_mode=DR)
                # Eviction: only the strip's first 128 cols can carry a
                # nonzero mask (diagonal/pad block of slot kt//2); they get
                # a DVE tensor_tensor add.  The rest are plain copies,
                # split ACT (kg0) / DVE (kg1) to halve eviction latency.
                # kg1 writes land on sT partitions 64:128 directly.
                # Full-width copy-out split ACT (kg0) / DVE (kg1); kg1
                # lands on sT partitions 64:128 directly.  The masked
                # first block then gets an in-place SBUF add on the
                # (otherwise idle) GpSimd engine, so the DVE carries only
                # one copy per chunk and psS keeps pace with the PE.
                nc.scalar.activation(
                    _strip_dst(kt, c0, c1)(0), ps[:, 0, :cw], COPY)
                nc.vector.tensor_copy(
                    _strip_dst(kt, c0, c1)(1), ps[:, 1, :cw])
                if c0 == 0:
                    for kg in range(2):
                        dst = _strip_dst(kt, 0, P)(kg)
                        nc.gpsimd.tensor_tensor(
                            dst, dst, masks[kt][kg * 64:(kg + 1) * 64, :],
                            op=mybir.AluOpType.add)
            # fire exp for every slot whose strips are all processed now:
            # slots 0..3 at their last strip (kt = CNT-1 in the second
            # part), slots 4..7 once strip 7 closes the second part.
            ready = [i for i in range(QT) if CNT[i] == kt + 1] if kt < 8 else []
            if kt == 7:
                ready += [i for i in range(QT) if CNT[i] > 8]
            for i in ready:
                pT_i = ptpool.tile([P, ST, P], BF16, tag="pt", name="pT_i")
                nc.scalar.activation(
                    pT_i[:, 0:min(CNT[i], 8), :],
                    sTB[:, 0:min(CNT[i], 8), i * P:(i + 1) * P],
                    EXP, scale=INV_SQRT_D)
                if CNT[i] > 8:
                    nc.scalar.activation(
                        pT_i[:, 8:CNT[i], :],
                        sTA[:, 0:CNT[i] - 8, (i - 4) * P:(i - 3) * P],
                        EXP, scale=INV_SQRT_D)
                pTs[i] = pT_i

        for i in range(QT):
            ck = CNT[i]
            pT_i = pTs[i]
            psavs = [psAV.tile([P, 512], F32, tag="psAV", name="psavs")
                     for _ in range(2)]
            for kt in range(ck):
                lhsT = pT_i[:, kt, :]
                nc.tensor.matmul(rs[:, i:i + 1], lhsT=lhsT, rhs=ones[:],
                                 start=(kt == 0), stop=(kt == ck - 1))
                for ec in range(2):
                    nc.tensor.matmul(
                        psavs[ec][:], lhsT=lhsT,
                        rhs=vsbc[kt % 8 // 4][:, kt // 8, kt % 4,
                                              ec * 512:(ec + 1) * 512],
                        start=(kt == 0), stop=(kt == ck - 1))

            recip = stpool.tile([P, 1], F32, tag="rc", name="recip")
            nc.vector.reciprocal(recip[:], rs[:, i:i + 1])
            for ec in range(2):
                o_t = opool.tile([P, 512], F32, tag="o", name="o_t")
                nc.scalar.activation(o_t[:], psavs[ec][:], COPY,
                                     scale=recip[:])
                nc.sync.dma_start(
                    out[i * P:(i + 1) * P, ec * 512:(ec + 1) * 512], o_t[:])


_COMPILED = None


def _get_compiled():
    global _COMPILED
    if _COMPILED is None:
        _COMPILED = _build()
    return _COMPILED


def _qrows(G):
    return np.concatenate([np.arange(g * P, (g + 1) * P) for g in G])


def _host_mask(G):
    # One [128,128] additive block per strip kt: the slot JKT[kt] block
    # (diagonal for one variant, full -NEG padding for the other; later
    # slots are strictly causal-active so their mask is identically 0).
    m = np.empty((P, ST * P), np.float32)
    for kt in range(ST):
        key = kt * P + np.arange(P)[:, None]
        qpos = G[JKT[kt]] * P + np.arange(P)[None, :]
        m[:, kt * P:(kt + 1) * P] = np.where(
            key <= qpos, np.float32(0.0), np.float32(NEG))
    return m.astype(ml_dtypes.bfloat16)


def _host_in_maps(X, Wq, Wk, Wv):
    bf = ml_dtypes.bfloat16
    X = np.asarray(X, np.float32)
    wq = np.asarray(Wq, np.float32).astype(bf)
    wk = np.asarray(Wk, np.float32).astype(bf)
    wv = np.asarray(Wv, np.float32).astype(bf)
    masks = {0: _host_mask(G_A), 1: _host_mask(G_B)}
    qr = {0: _qrows(G_A), 1: _qrows(G_B)}
    in_maps = []
    for c in range(NCORES):
        b, h = divmod(c, 2)
        Xb = X[b]
        in_maps.append({
            "xt": np.ascontiguousarray(Xb[h * SLOC:(h + 1) * SLOC].T).astype(bf),
            "xqt": np.ascontiguousarray(Xb[qr[h]].T).astype(bf),
            "wq": wq, "wk": wk, "wv": wv,
            "mask": masks[h],
        })
    return in_maps, qr


def kernel(X, Wq, Wk, Wv, _trace=False):
    nc = _get_compiled()
    in_maps, qr = _host_in_maps(X, Wq, Wk, Wv)
    res = run_bass_kernel_spmd(nc, in_maps, core_ids=list(range(NCORES)),
                               trace=_trace)
    O = np.empty((B, S, D), np.float32)
    for c in range(NCORES):
        b, h = divmod(c, 2)
        O[b, qr[h]] = res.results[c]["out"]
    if _trace:
        kernel._last_exec_time_ns = res.exec_time_ns
        kernel._last_results = res
    return O


# revision 38
# speedup vs baseline: 1.2735x; 1.0089x over previous
"""Causal attention (B=4, S=2048, D=1024, fp32 in/out) on 8 Trainium2 cores.

Sharding: core c = (batch b = c//2, variant h = c%2). Each core computes the
attention output for 1024 of the 2048 query rows of one batch element.

Load balancing ("parity-slot" assignment): variant A owns even global
q-tiles (0,2,...,14), variant B owns odd (1,3,...,15). Slot i on every core
processes keys [0, CNT[i]*128) with CNT = (2,4,6,...,16), which dominates
both variants' causal needs (72 key-tiles vs the 68 minimum), so a single
NEFF serves all 8 cores; per-core differences are carried entirely by input
data (pre-sliced/pre-transposed X, per-strip diagonal mask blocks).

A consequence of the parity assignment: for score strip kt, ONLY the first
128-column slot block (slot JKT[kt] = kt//2) can have a nonzero causal
mask (diagonal for one variant, fully-masked padding for the other); all
later slot blocks are strictly below the diagonal for both variants. So
the mask input is just one [128,128] block per strip, and all remaining
eviction columns are plain copies, split across the ACT and DVE engines.

K/V are not recomputed per core: core (b, h) projects K^T/V only for its
own key half [h*1024, (h+1)*1024), and the pair exchanges halves with
chunked AllGathers over replica groups [[0,1],[2,3],[4,5],[6,7]] through
DRAM bounce buffers with partition-contiguous rows (4-8KB per partition,
fast DMA), pipelined so early key tiles land in SBUF while later
projection halves still compute.  No warm-up collective: the runtime's
collectives-init barrier occupies the CC stream until ~40us regardless.

Numerics: projections and AV run in bf16 (fp32 PSUM accum). Scores run in
fp8e4 (e4m3) with DoubleRow perf mode - each matmul contracts TWO 128-e
tiles into 64 psum partitions at 0.5 cycles/column, 2x bf16 throughput.
Q^T/K^T are cast fp32->fp8 at projection eviction; the 1/sqrt(1024) logit
scale is applied inside the exp activation (scale=1/32), so q/k stay at
full range where e4m3 quantization is benign. Measured end-to-end rel err
~1.3e-2 (vs 2e-2 budget).

DoubleRow cannot target PSUM partition offset 64 (invalid ISA), so the two
64-key groups of a strip go to separate psum regions at partition base 0;
the eviction writes group 1 to sT partitions 64:128 directly (engines
honor per-operand partition bases).

Tile-granularity dependencies: the Tile framework serializes readers
behind ALL writers of a tile, so every cross-phase tensor is split into
per-chunk tiles (qT per 512-q half, kT/v_sb per gather chunk, sT per
strip-group) to make phase overlap real: score strips 8..15 touch only
the qc=1 half of Q^T and start while the qc=0 half is still projecting.
"""

import numpy as np
from contextlib import ExitStack

import ml_dtypes

import concourse.bass as bass
import concourse.tile as tile
from concourse import bacc, mybir
from concourse.bass_utils import run_bass_kernel_spmd

P = 128
B, S, D = 4, 2048, 1024
NCORES = 8
DT = D // P      # 8 contraction tiles
ST = S // P      # 16 key tiles (global)
SLOC = S // 2    # 1024 local keys per core
SLT = SLOC // P  # 8 local key tiles
ET = D // P      # 8 output-feature tiles
QLOC = 1024      # query rows per core
QT = QLOC // P   # 8 local q tiles

G_A = tuple(range(0, ST, 2))         # variant A global q-tiles (slot order)
G_B = tuple(range(1, ST, 2))         # variant B
CNT = tuple(2 * i + 2 for i in range(QT))  # key tiles per slot (shared)
# Scores are computed transposed (S^T[k, q], keys on partitions).  Because
# CNT is ascending, the slots active for key-tile kt form a contiguous
# q-suffix starting at slot JKT[kt] = kt//2; WKT[kt] is that suffix width.
JKT = tuple(kt // 2 for kt in range(ST))
WKT = tuple((QT - j) * P for j in JKT)
NEG = -10000.0
INV_SQRT_D = 1.0 / 32.0
# Score strips 8..15 first (they only need the qc=1 half of Q^T), then
# 0..7.  Slots 0..3 finish at strip CNT[i]-1 in the second part; slots
# 4..7 need strips from both parts and all finish after strip 7.
STRIP_ORDER = tuple(range(8, ST)) + tuple(range(8))

F32 = mybir.dt.float32
BF16 = mybir.dt.bfloat16
F8 = mybir.dt.float8e4
DR = mybir.MatmulPerfMode.DoubleRow
EXP = mybir.ActivationFunctionType.Exp
COPY = mybir.ActivationFunctionType.Copy

REPLICA_GROUPS = [[0, 1], [2, 3], [4, 5], [6, 7]]


def _build(reps=1):
    nc = bacc.Bacc("TRN2", target_bir_lowering=False, debug=False,
                   num_devices=NCORES)
    xt_in = nc.dram_tensor("xt", [D, SLOC], BF16, kind="ExternalInput").ap()
    xqt_in = nc.dram_tensor("xqt", [D, QLOC], BF16, kind="ExternalInput").ap()
    wq_in = nc.dram_tensor("wq", [D, D], BF16, kind="ExternalInput").ap()
    wk_in = nc.dram_tensor("wk", [D, D], BF16, kind="ExternalInput").ap()
    wv_in = nc.dram_tensor("wv", [D, D], BF16, kind="ExternalInput").ap()
    mask_in = nc.dram_tensor("mask", [P, ST * P], BF16,
                             kind="ExternalInput").ap()
    out = nc.dram_tensor("out", [QLOC, D], F32, kind="ExternalOutput").ap()

    with tile.TileContext(nc) as tc, ExitStack() as ctx:
        persist = ctx.enter_context(tc.tile_pool(name="persist", bufs=1))
        # K^T per key chunk: [e%128, rank, et, key%512]; chunk kc covers
        # local key cols [kc*512,(kc+1)*512) of both ranks.
        kTc = [persist.tile([P, 2, ET, 512], F8, tag=f"kT{c}", name=f"kT{c}")
               for c in range(2)]
        # Q^T per 512-query half: [e%128, et, q%512]
        qTh = [persist.tile([P, ET, 512], F8, tag=f"qT{c}", name=f"qT{c}")
               for c in range(2)]
        # V per gather chunk: [k%128, rank, local kt%4, e]; chunk c covers
        # local key tiles [4c, 4c+4) of both ranks.
        vsbc = [persist.tile([P, 2, 4, D], BF16, tag=f"v{c}", name=f"v{c}")
                for c in range(2)]
        ones = persist.tile([P, 1], BF16, tag="ones")

        for _rep in range(reps):
            _emit_body(nc, tc, _rep, xt_in, xqt_in, wq_in, wk_in, wv_in,
                       mask_in, out, kTc, qTh, vsbc, ones)
    nc.compile()
    return nc


def _emit_body(nc, tc, rep, xt_in, xqt_in, wq_in, wk_in, wv_in, mask_in, out,
               kTc, qTh, vsbc, ones):
    body = ExitStack()
    # Per-strip [128,128] diagonal mask blocks; tiny, prefetch all 16.
    mpool = body.enter_context(tc.tile_pool(name="m", bufs=ST))
    masks = {}

    def _load_mask(kt):
        m_t = mpool.tile([P, P], BF16, tag="m", name="m_t")
        nc.sync.dma_start(m_t, mask_in[:, kt * P:(kt + 1) * P])
        masks[kt] = m_t

    # ---------------- Phase A : projections + KV exchange ----------------
    with ExitStack() as pa:
        xp = pa.enter_context(tc.tile_pool(name="xp", bufs=1))
        dp = pa.enter_context(tc.tile_pool(name="dp", bufs=1, space="DRAM"))
        psA = pa.enter_context(tc.tile_pool(name="psA", bufs=8, space="PSUM"))

        nc.gpsimd.memset(ones[:], 1.0)

        # K-proj inputs (wk+xt) split across BOTH DMA queues so the first
        # matmul starts after ~0.5MB and per-dt delivery outpaces the PE.
        xt = xp.tile([P, DT, SLOC], BF16, tag="xt")
        wq_t = xp.tile([P, DT, D], BF16, tag="wq")
        wk_t = xp.tile([P, DT, D], BF16, tag="wk")
        wv_t = xp.tile([P, DT, D], BF16, tag="wv")
        xqt = xp.tile([P, DT, QLOC], BF16, tag="xqt")
        for dt in range(DT):
            nc.sync.dma_start(wk_t[:, dt, :], wk_in[dt * P:(dt + 1) * P, :])
            nc.scalar.dma_start(xt[:, dt, :], xt_in[dt * P:(dt + 1) * P, :])
        for dt in range(DT):
            nc.sync.dma_start(wv_t[:, dt, :], wv_in[dt * P:(dt + 1) * P, :])
        for dt in range(DT):
            nc.scalar.dma_start(xqt[:, dt, :], xqt_in[dt * P:(dt + 1) * P, :])
        for dt in range(DT):
            nc.scalar.dma_start(wq_t[:, dt, :], wq_in[dt * P:(dt + 1) * P, :])
        for kt in range(ST):
            _load_mask(kt)

        # Bounce layouts are partition-contiguous (4-8KB per partition
        # row), so stores/loads are single fast DMAs, not strided scatter.
        klocal = xp.tile([P, 2, ET, 512], F8, tag="klocal")
        vlocal = xp.tile([P, 2, 4, D], BF16, tag="vlocal")
        kbounce = [dp.tile([P, ET * 512], F8, tag="kb", name=f"kb{c}")
                   for c in range(2)]
        kgather = [dp.tile([2 * P, ET * 512], F8, tag="kg", name=f"kg{c}")
                   for c in range(2)]
        vbounce = [dp.tile([P, 4 * D], BF16, tag="vb", name=f"vb{c}")
                   for c in range(2)]
        vgather = [dp.tile([2 * P, 4 * D], BF16, tag="vg", name=f"vg{c}")
                   for c in range(2)]

        # K^T_loc[et, k] = sum_d Wk[d, et].T X_loc^T[d, k].  Halves are key
        # chunks (kc), so chunk kc's store+gather overlaps the other half's
        # matmuls; dt is the outer loop so matmuls start as slices land.
        for kc in range(2):
            pss = [psA.tile([P, 512], F32, tag="ps", name="ps")
                   for _ in range(ET)]
            for dt in range(DT):
                for et in range(ET):
                    nc.tensor.matmul(
                        pss[et][:], lhsT=wk_t[:, dt, et * P:(et + 1) * P],
                        rhs=xt[:, dt, kc * 512:(kc + 1) * 512],
                        start=(dt == 0), stop=(dt == DT - 1))
            for et in range(ET):
                nc.vector.tensor_copy(klocal[:, kc, et, :], pss[et][:])
            nc.gpsimd.dma_start(
                kbounce[kc].rearrange("p (et k) -> p et k", et=ET),
                klocal[:, kc, :, :])
            nc.gpsimd.collective_compute(
                "AllGather", mybir.AluOpType.bypass,
                replica_groups=REPLICA_GROUPS,
                ins=[kbounce[kc].opt()], outs=[kgather[kc].opt()])
            # Gather-dependent loads go on the sync queue: the scalar
            # queue carries the phase-B ACT psS eviction copies, which a
            # waiting DMA trigger would head-of-line block behind the
            # V collective.
            for r in range(2):
                nc.sync.dma_start(
                    kTc[kc][:, r, :, :],
                    kgather[kc][r * P:(r + 1) * P, :].rearrange(
                        "p (et k) -> p et k", et=ET))

        # V_loc[kt, e] = sum_d X_loc^T[d, kt].T Wv[d, e]; each half (4 local
        # key tiles) is one store+gather chunk overlapping later compute.
        for half in range(2):
            groups = [(st, ec) for st in range(4) for ec in range(2)]
            pss = [psA.tile([P, 512], F32, tag="ps", name="ps")
                   for _ in groups]
            for dt in range(DT):
                for gi, (st, ec) in enumerate(groups):
                    nc.tensor.matmul(
                        pss[gi][:],
                        lhsT=xt[:, dt, (half * 4 + st) * P:(half * 4 + st + 1) * P],
                        rhs=wv_t[:, dt, ec * 512:(ec + 1) * 512],
                        start=(dt == 0), stop=(dt == DT - 1))
            for gi, (st, ec) in enumerate(groups):
                nc.vector.tensor_copy(
                    vlocal[:, half, st, ec * 512:(ec + 1) * 512], pss[gi][:])
            nc.gpsimd.dma_start(
                vbounce[half].rearrange("p (st e) -> p st e", st=4),
                vlocal[:, half, :, :])
            nc.gpsimd.collective_compute(
                "AllGather", mybir.AluOpType.bypass,
                replica_groups=REPLICA_GROUPS,
                ins=[vbounce[half].opt()], outs=[vgather[half].opt()])
            for r in range(2):
                nc.sync.dma_start(
                    vsbc[half][:, r, :, :],
                    vgather[half][r * P:(r + 1) * P, :].rearrange(
                        "p (st e) -> p st e", st=4))

        # Q^T[et, q] = sum_d Wq[d, et].T Xq^T[d, q].  Halves are q chunks,
        # qc=1 FIRST: score strips 8..15 touch only q-cols [512:1024), so
        # they start as soon as the qc=1 half is evicted, overlapping the
        # qc=0 half and hiding the Q->scores transition.
        for qc in (1, 0):
            pss = [psA.tile([P, 512], F32, tag="ps", name="ps")
                   for _ in range(ET)]
            for dt in range(DT):
                for et in range(ET):
                    nc.tensor.matmul(
                        pss[et][:], lhsT=wq_t[:, dt, et * P:(et + 1) * P],
                        rhs=xqt[:, dt, qc * 512:(qc + 1) * 512],
                        start=(dt == 0), stop=(dt == DT - 1))
            for et in range(ET):
                nc.vector.tensor_copy(qTh[qc][:, et, :], pss[et][:])

    # ---------------- Phase B : attention (transposed scores) ----------
    # S^T[k, q] with keys on partitions, fp8 DoubleRow: each matmul
    # contracts an et PAIR into 64 psum partitions (one 64-key group).
    # exp(S^T) directly yields P^T -- the AV stationary operand.
    with body, ExitStack() as pb:
        stile = pb.enter_context(tc.tile_pool(name="st", bufs=1))
        # sT split: strips 8..15 span only q-cols [512:1024) -> 512 wide.
        sTA = stile.tile([P, 8, 512], F32, tag="sTA")    # strips 8..15
        sTB = stile.tile([P, 8, QLOC], F32, tag="sTB")   # strips 0..7
        # per-slot P^T tiles so an early slot's AV only waits its own exp
        ptpool = pb.enter_context(tc.tile_pool(name="pt", bufs=QT))
        opool = pb.enter_context(tc.tile_pool(name="o", bufs=2))
        stpool = pb.enter_context(tc.tile_pool(name="stat", bufs=QT))
        psS = pb.enter_context(tc.tile_pool(name="psS", bufs=2, space="PSUM"))
        psAV = pb.enter_context(tc.tile_pool(name="psAV", bufs=3, space="PSUM"))
        psRS = pb.enter_context(tc.tile_pool(name="psRS", bufs=1, space="PSUM"))
        rs = psRS.tile([P, QT], F32, tag="rs")           # rowsum, col per slot

        def _strip_dst(kt, s0, s1):
            # sT slice of strip kt covering strip-local cols [s0:s1),
            # returned as fn(kg) -> [64, s1-s0] AP at partitions kg*64.
            jq = JKT[kt] * P
            if kt >= 8:
                return lambda kg: sTA[kg * 64:(kg + 1) * 64, kt - 8,
                                      jq - 512 + s0:jq - 512 + s1]
            return lambda kg: sTB[kg * 64:(kg + 1) * 64, kt,
                                  jq + s0:jq + s1]

        pTs = {}
        for kt in STRIP_ORDER:
            jq = JKT[kt] * P
            w = WKT[kt]
            # chunks aligned to the global 512-q grid so each chunk's rhs
            # lives in exactly one qTh tile
            if jq >= 512:
                chunks = [(0, w)]
            else:
                chunks = [(0, 512 - jq), (512 - jq, w)]
            for c0, c1 in chunks:
                cw = c1 - c0
                qc = (jq + c0) // 512
                qoff = (jq + c0) - qc * 512
                ps = psS.tile([64, 2, 512], F32, tag="psS", name="ps")
                # ep-outer: the two kg chains interleave so back-to-back
                # matmuls never accumulate into the same psum region
                for ep in range(ET // 2):
                    for kg in range(2):
                        lo = kt % 4 * P + kg * 64
                        nc.tensor.matmul(
                            ps[:, kg, :cw],
                            lhsT=kTc[kt % 8 // 4][:, kt // 8,
                                                  2 * ep:2 * ep + 2,
                                                  lo:lo + 64],
                            rhs=qTh[qc][:, 2 * ep:2 * ep + 2,
                                        qoff:qoff + cw],
                            start=(ep == 0), stop=(ep == ET // 2 - 1),
                            perf_mode=DR)
                # Eviction: only the strip's first 128 cols can carry a
                # nonzero mask (diagonal/pad block of slot kt//2); they get
                # a DVE tensor_tensor add.  The rest are plain copies,
                # split ACT (kg0) / DVE (kg1) to halve eviction latency.
                # kg1 writes land on sT partitions 64:128 directly.
                # Full-width copy-out split ACT (kg0) / DVE (kg1); kg1
                # lands on sT partitions 64:128 directly.  The masked
                # first block then gets an in-place SBUF add on the
                # (otherwise idle) GpSimd engine, so the DVE carries only
                # one copy per chunk and psS keeps pace with the PE.
                nc.scalar.activation(
                    _strip_dst(kt, c0, c1)(0), ps[:, 0, :cw], COPY)
                nc.vector.tensor_copy(
                    _strip_dst(kt, c0, c1)(1), ps[:, 1, :cw])
                if c0 == 0:
                    for kg in range(2):
                        dst = _strip_dst(kt, 0, P)(kg)
                        nc.gpsimd.tensor_tensor(
                            dst, dst, masks[kt][kg * 64:(kg + 1) * 64, :],
                            op=mybir.AluOpType.add)
            # fire exp for every slot whose strips are all processed now:
            # slots 0..3 at their last strip (kt = CNT-1 in the second
            # part), slots 4..7 once strip 7 closes the second part.
            ready = [i for i in range(QT) if CNT[i] == kt + 1] if kt < 8 else []
            if kt == 7:
                ready += [i for i in range(QT) if CNT[i] > 8]
            for i in ready:
                pT_i = ptpool.tile([P, ST, P], BF16, tag="pt", name="pT_i")
                nc.scalar.activation(
                    pT_i[:, 0:min(CNT[i], 8), :],
                    sTB[:, 0:min(CNT[i], 8), i * P:(i + 1) * P],
                    EXP, scale=INV_SQRT_D)
                if CNT[i] > 8:
                    nc.scalar.activation(
                        pT_i[:, 8:CNT[i], :],
                        sTA[:, 0:CNT[i] - 8, (i - 4) * P:(i - 3) * P],
                        EXP, scale=INV_SQRT_D)
                pTs[i] = pT_i

        for i in range(QT):
            ck = CNT[i]
            pT_i = pTs[i]
            psavs = [psAV.tile([P, 512], F32, tag="psAV", name="psavs")
                     for _ in range(2)]
            for kt in range(ck):
                lhsT = pT_i[:, kt, :]
                nc.tensor.matmul(rs[:, i:i + 1], lhsT=lhsT, rhs=ones[:],
                                 start=(kt == 0), stop=(kt == ck - 1))
                for ec in range(2):
                    nc.tensor.matmul(
                        psavs[ec][:], lhsT=lhsT,
                        rhs=vsbc[kt % 8 // 4][:, kt // 8, kt % 4,
                                              ec * 512:(ec + 1) * 512],
                        start=(kt == 0), stop=(kt == ck - 1))

            recip = stpool.tile([P, 1], F32, tag="rc", name="recip")
            nc.vector.reciprocal(recip[:], rs[:, i:i + 1])
            for ec in range(2):
                o_t = opool.tile([P, 512], F32, tag="o", name="o_t")
                nc.scalar.activation(o_t[:], psavs[ec][:], COPY,
                                     scale=recip[:])
                nc.sync.dma_start(
                    out[i * P:(i + 1) * P, ec * 512:(ec + 1) * 512], o_t[:])


_COMPILED = None


def _get_compiled():
    global _COMPILED
    if _COMPILED is None:
        _COMPILED = _build()
    return _COMPILED


def _qrows(G):
    return np.concatenate([np.arange(g * P, (g + 1) * P) for g in G])


def _host_mask(G):
    # One [128,128] additive block per strip kt: the slot JKT[kt] block
    # (diagonal for one variant, full -NEG padding for the other; later
    # slots are strictly causal-active so their mask is identically 0).
    m = np.empty((P, ST * P), np.float32)
    for kt in range(ST):
        key = kt * P + np.arange(P)[:, None]
        qpos = G[JKT[kt]] * P + np.arange(P)[None, :]
        m[:, kt * P:(kt + 1) * P] = np.where(
            key <= qpos, np.float32(0.0), np.float32(NEG))
    return m.astype(ml_dtypes.bfloat16)


def _host_in_maps(X, Wq, Wk, Wv):
    bf = ml_dtypes.bfloat16
    X = np.asarray(X, np.float32)
    wq = np.asarray(Wq, np.float32).astype(bf)
    wk = np.asarray(Wk, np.float32).astype(bf)
    wv = np.asarray(Wv, np.float32).astype(bf)
    masks = {0: _host_mask(G_A), 1: _host_mask(G_B)}
    qr = {0: _qrows(G_A), 1: _qrows(G_B)}
    in_maps = []
    for c in range(NCORES):
        b, h = divmod(c, 2)
        Xb = X[b]
        in_maps.append({
            "xt": np.ascontiguousarray(Xb[h * SLOC:(h + 1) * SLOC].T).astype(bf),
            "xqt": np.ascontiguousarray(Xb[qr[h]].T).astype(bf),
            "wq": wq, "wk": wk, "wv": wv,
            "mask": masks[h],
        })
    return in_maps, qr


def kernel(X, Wq, Wk, Wv, _trace=False):
    nc = _get_compiled()
    in_maps, qr = _host_in_maps(X, Wq, Wk, Wv)
    res = run_bass_kernel_spmd(nc, in_maps, core_ids=list(range(NCORES)),
                               trace=_trace)
    O = np.empty((B, S, D), np.float32)
    for c in range(NCORES):
        b, h = divmod(c, 2)
        O[b, qr[h]] = res.results[c]["out"]
    if _trace:
        kernel._last_exec_time_ns = res.exec_time_ns
        kernel._last_results = res
    return O
